# revision 3
# baseline (speedup 1.0000x reference)
"""GQA (16 q-heads / 4 kv-heads, D=128, S=2048, E=2048, B=2) on 8 trn2 cores.

Sharding: core = 4*b + g  (b in {0,1} batch, g in {0..3} kv-head group).
Each core computes its batch's 4 query heads (one kv group) end-to-end.

v3 design (v1 baseline 432us, v2 362us):
 - bf16 attention operands (qt/kt/at/vn; numpy-verified rel err 0.42%).
 - Consolidated DMAs, host pre-permuted to [partition, chunk, free]; load
   order tuned so the PE is gated only ~4us at kernel start (wk/x0
   quarters interleaved, per-head wq split, x before rope tables).
 - V projected directly into natural [s, d] layout (x-tile stationary x
   bf16 wv moving), no PE transposes.
 - Unified qk tile [D, 5, S] (slot 0 = K, 1+h = Q head h): one batched
   rotate-half partition swap (2 DMAs/chunk on the Act queue) and 3 wide
   [128,5,512] bf16 combines on DVE with stride-0 broadcast cos/sin APs.
 - Scores matmuls fill [128,2,512] 2-bank PSUM pairs; ONE wide exp (1024
   free) per pair into contiguous bf16 at[128,16,512].
 - Softmax denominator entirely off PE: wide bf16 tensor_tensor tree on
   DVE, gpsimd partition_all_reduce, DVE reciprocal, gpsimd mult.
 - o_proj(q) interleaved after attn(q+1); last chunk's PSUM drains via
   the idle Act engine; output rows DMA'd in halves as produced.
"""

import numpy as np
import ml_dtypes

import concourse.bass as bass
import concourse.bacc as bacc
import concourse.mybir as mybir
import concourse.tile as tile
from concourse import bass_isa
from concourse.ap import AP
from concourse.bass_utils import run_bass_kernel_spmd

B, S, E = 2, 2048, 2048
H, HKV, D = 16, 4, 128
G = H // HKV          # 4 query heads per kv group
GD = G * D            # 512 channels per group
NCORES = 8
SCALE = 1.0 / float(np.sqrt(D))
ROPE_BASE = 10000.0

NE = E // 128         # 16 e-chunks (contraction for projections)
NSC = S // 512        # 4 s-chunks of 512
NST = S // 128        # 16 s-tiles of 128

F32 = mybir.dt.float32
F32R = mybir.dt.float32r
BF16 = mybir.dt.bfloat16
AF = mybir.ActivationFunctionType
OP = mybir.AluOpType


def _r(ap):
    return ap.bitcast(F32R)


def _bcast_mid(ap2d, n):
    """[P, F] AP -> [P, n, F] AP with stride-0 middle dim (broadcast)."""
    dims = [list(x) for x in ap2d.ap]
    return AP(ap2d.tensor, ap2d.offset, [dims[0], [0, n], dims[1]])


def _emit(nc, tc, xT, wq, wk, wv, wo, cosT, sinTf, out):
    from contextlib import ExitStack
    es = ExitStack()
    with es:
        gpool = es.enter_context(tc.tile_pool(name="glob", bufs=1))
        # slot 0 = K, slots 1..4 = Q heads (transposed [d, s] layout, bf16)
        qk_sb = gpool.tile([D, 5, S], BF16, tag="qk")
        vn_sb = gpool.tile([128, NST, D], BF16, tag="vn")
        wo_sb = gpool.tile([128, G, E], F32, tag="wo")

        # ================= phase A: projections + RoPE =================
        with (
            tc.tile_pool(name="phA", bufs=1) as pa,
            tc.tile_pool(name="xs", bufs=2) as xpool,
            tc.tile_pool(name="ropetmp", bufs=2) as rpool,
            tc.tile_pool(name="psA", bufs=1, space=bass.MemorySpace.PSUM) as psA,
        ):
            wk_sb = pa.tile([128, NE, D], F32, tag="wk")
            cos_sb = pa.tile([D, S], BF16, tag="cos")
            sin_sb = pa.tile([D, S], BF16, tag="sin")
            wv_sb = pa.tile([128, NE, D], BF16, tag="wv")
            wq_sb = pa.tile([128, NE, GD], F32, tag="wq")

            xsl = [xpool.tile([128, NE, 512], F32, tag="xs", name=f"xs{q}")
                   for q in range(NSC)]
            # interleave wk and x0 quarters so K accumulation starts ASAP
            for qq in range(4):
                nc.sync.dma_start(out=wk_sb[:, 4 * qq:4 * qq + 4, :],
                                  in_=wk.ap()[:, 4 * qq:4 * qq + 4, :])
                nc.sync.dma_start(out=xsl[0][:, 4 * qq:4 * qq + 4, :],
                                  in_=xT.ap()[:, 4 * qq:4 * qq + 4, 0:512])
            nc.sync.dma_start(out=wv_sb[:], in_=wv.ap())
            nc.sync.dma_start(out=wq_sb[:, :, 0:D], in_=wq.ap()[:, :, 0:D])
            nc.sync.dma_start(out=wq_sb[:, :, D:GD], in_=wq.ap()[:, :, D:GD])
            nc.sync.dma_start(out=xsl[1][:], in_=xT.ap()[:, :, 512:1024])
            nc.sync.dma_start(out=xsl[2][:], in_=xT.ap()[:, :, 1024:1536])
            nc.sync.dma_start(out=cos_sb[:], in_=cosT.ap())
            nc.sync.dma_start(out=sin_sb[:], in_=sinTf.ap())
            nc.sync.dma_start(out=xsl[3][:], in_=xT.ap()[:, :, 1536:2048])
            nc.sync.dma_start(out=wo_sb[:], in_=wo.ap())

            for q in range(NSC):
                sl = slice(q * 512, (q + 1) * 512)
                x = xsl[q]
                qraw = rpool.tile([128, 5, 512], BF16, tag="qraw")
                qswp = rpool.tile([128, 5, 512], BF16, tag="qswp")
                rot = rpool.tile([128, 5, 512], BF16, tag="rot")
                # K projection (transposed layout)
                ps = psA.tile([128, 512], F32, tag="proj", bufs=2)
                for j in range(NE):
                    nc.tensor.matmul(ps[:], _r(wk_sb[:, j, :]), _r(x[:, j, :]),
                                     start=(j == 0), stop=(j == NE - 1))
                nc.vector.tensor_copy(qraw[:, 0, :], ps[:])
                # V projection directly into natural [s, d] layout
                psv = psA.tile([128, 4, D], F32, tag="vproj", bufs=2)
                for st in range(4):
                    t = q * 4 + st
                    ssl128 = slice(st * 128, (st + 1) * 128)
                    for j in range(NE):
                        nc.tensor.matmul(psv[:, st, :], _r(x[:, j, ssl128]),
                                         wv_sb[:, j, :],
                                         start=(j == 0), stop=(j == NE - 1))
                    nc.vector.tensor_copy(vn_sb[:, t, :], psv[:, st, :])
                # Q projections
                for h in range(G):
                    ps = psA.tile([128, 512], F32, tag="proj", bufs=2)
                    for j in range(NE):
                        nc.tensor.matmul(ps[:], _r(wq_sb[:, j, h * D:(h + 1) * D]),
                                         _r(x[:, j, :]),
                                         start=(j == 0), stop=(j == NE - 1))
                    nc.vector.tensor_copy(qraw[:, 1 + h, :], ps[:])
                # batched rope for all 5 projections of this chunk:
                # partition swap via 2 DMAs (Act queue; Act idle in phase A)
                nc.scalar.dma_start(out=qswp[0:64, :, :], in_=qraw[64:128, :, :])
                nc.scalar.dma_start(out=qswp[64:128, :, :], in_=qraw[0:64, :, :])
                cb = _bcast_mid(cos_sb[:, sl], 5)
                sb = _bcast_mid(sin_sb[:, sl], 5)
                nc.vector.tensor_tensor(rot[:], qswp[:], sb, OP.mult)
                nc.vector.tensor_tensor(qraw[:], qraw[:], cb, OP.mult)
                nc.vector.tensor_tensor(qk_sb[:, :, sl], qraw[:], rot[:], OP.add)

        # ================= phase B+C: attention + o_proj interleaved ====
        with (
            tc.tile_pool(name="atp", bufs=2) as atpool,
            tc.tile_pool(name="otp", bufs=2) as otpool,
            tc.tile_pool(name="nrm", bufs=2) as nrmpool,
            tc.tile_pool(name="ost", bufs=2) as opool,
            tc.tile_pool(name="psB", bufs=1, space=bass.MemorySpace.PSUM) as psB,
        ):
            ot_tiles = {}

            def attn_iter(q, h):
                sl = slice(q * 512, (q + 1) * 512)
                at = atpool.tile([128, NST, 512], BF16, tag="at")
                av = psB.tile([D, 512], F32, tag="av", bufs=2)
                for tg in range(8):
                    sc2 = psB.tile([128, 2, 512], F32, tag="sc", bufs=2)
                    for tt in range(2):
                        t = 2 * tg + tt
                        nc.tensor.matmul(sc2[:, tt, :],
                                         qk_sb[:, 0, t * 128:(t + 1) * 128],
                                         qk_sb[:, 1 + h, sl],
                                         start=True, stop=True)
                    nc.scalar.activation(at[:, 2 * tg:2 * tg + 2, :], sc2[:],
                                         AF.Exp, scale=SCALE)
                    for tt in range(2):
                        t = 2 * tg + tt
                        nc.tensor.matmul(av[:], vn_sb[:, t, :], at[:, t, :],
                                         start=(t == 0), stop=(t == NST - 1))
                ot = otpool.tile([D, 512], F32, tag=f"ot{h}", name=f"ot{h}_{q}")
                ot_tiles[(q, h)] = ot
                nc.vector.tensor_copy(ot[:], av[:])
                # denominator: wide bf16 pairwise tree on DVE (in-place),
                # then cross-partition sum + broadcast on gpsimd
                with nc.allow_low_precision(reason="bf16 softmax denom, verified 4e-3 rel err"):
                    nc.vector.tensor_tensor(at[:, 0:8, :], at[:, 0:8, :],
                                            at[:, 8:16, :], OP.add)
                    nc.vector.tensor_tensor(at[:, 0:4, :], at[:, 0:4, :],
                                            at[:, 4:8, :], OP.add)
                    nc.vector.tensor_tensor(at[:, 0:2, :], at[:, 0:2, :],
                                            at[:, 2:4, :], OP.add)
                    acc = nrmpool.tile([128, 512], BF16, tag="acc")
                    nc.vector.tensor_tensor(acc[:], at[:, 0, :], at[:, 1, :],
                                            OP.add)
                    den = nrmpool.tile([128, 512], F32, tag="den")
                    nc.gpsimd.partition_all_reduce(den[:], acc[:], 128,
                                                   bass_isa.ReduceOp.add)
                    rc = nrmpool.tile([128, 512], F32, tag="rc")
                    nc.vector.reciprocal(rc[:], den[:])
                nc.gpsimd.tensor_tensor(ot[:], ot[:], rc[:], OP.mult)

            def oproj(q, last=False):
                for st in range(4):
                    s0 = q * 512 + st * 128
                    ostg = opool.tile([128, E], F32, tag="ostg")
                    for eo in range(4):
                        op_ps = psB.tile([128, 512], F32, tag="op", bufs=2)
                        for h in range(G):
                            nc.tensor.matmul(
                                op_ps[:],
                                _r(ot_tiles[(q, h)][:, st * 128:(st + 1) * 128]),
                                _r(wo_sb[:, h, eo * 512:(eo + 1) * 512]),
                                start=(h == 0), stop=(h == G - 1))
                        osl = slice(eo * 512, (eo + 1) * 512)
                        if last:
                            nc.scalar.copy(ostg[:, osl], op_ps[:])
                        else:
                            nc.vector.tensor_copy(ostg[:, osl], op_ps[:])
                        if eo == 1:
                            nc.sync.dma_start(out=out.ap()[s0:s0 + 128, 0:1024],
                                              in_=ostg[:, 0:1024])
                        elif eo == 3:
                            nc.sync.dma_start(out=out.ap()[s0:s0 + 128, 1024:2048],
                                              in_=ostg[:, 1024:2048])

            for q in range(NSC):
                for h in range(G):
                    attn_iter(q, h)
                if q >= 1:
                    oproj(q - 1)
            oproj(NSC - 1, last=True)


def _build():
    nc = bacc.Bacc("TRN2", target_bir_lowering=False, debug=False,
                   num_devices=NCORES)
    xT = nc.dram_tensor("xT", [128, NE, S], F32, kind="ExternalInput")
    wq = nc.dram_tensor("wq", [128, NE, GD], F32, kind="ExternalInput")
    wk = nc.dram_tensor("wk", [128, NE, D], F32, kind="ExternalInput")
    wv = nc.dram_tensor("wv", [128, NE, D], BF16, kind="ExternalInput")
    wo = nc.dram_tensor("wo", [128, G, E], F32, kind="ExternalInput")
    cosT = nc.dram_tensor("cosT", [D, S], BF16, kind="ExternalInput")
    sinTf = nc.dram_tensor("sinTf", [D, S], BF16, kind="ExternalInput")
    out = nc.dram_tensor("out", [S, E], F32, kind="ExternalOutput")
    with tile.TileContext(nc) as tc:
        _emit(nc, tc, xT, wq, wk, wv, wo, cosT, sinTf, out)
    nc.compile()
    return nc


def _rope_tables():
    inv = 1.0 / (ROPE_BASE ** (np.arange(0, D, 2, dtype=np.float64) / D))
    t = np.arange(S, dtype=np.float64)
    freqs = t[:, None] * inv[None, :]                    # [S, D/2]
    emb = np.concatenate([freqs, freqs], axis=-1)        # [S, D]
    cosT = np.cos(emb).T.astype(np.float32)              # [D, S]
    sinT = np.sin(emb).T.astype(np.float32)
    sinTf = sinT.copy()
    sinTf[: D // 2] *= -1.0                              # fold rotate_half sign
    return (np.ascontiguousarray(cosT).astype(ml_dtypes.bfloat16),
            np.ascontiguousarray(sinTf).astype(ml_dtypes.bfloat16))


def _chunked(a, nchunk):
    """[E, F] -> [128, nchunk, F] with chunk c holding rows c*128..(c+1)*128."""
    E_, F_ = a.shape
    return np.ascontiguousarray(
        a.reshape(nchunk, 128, F_).transpose(1, 0, 2))


_NC = None
LAST_RESULTS = None


def kernel(hidden_states, wq, wk, wv, wo):
    global _NC, LAST_RESULTS
    if _NC is None:
        _NC = _build()
    cosT, sinTf = _rope_tables()
    hs = np.asarray(hidden_states, dtype=np.float32)
    wq = np.asarray(wq, dtype=np.float32)
    wk = np.asarray(wk, dtype=np.float32)
    wv = np.asarray(wv, dtype=np.float32)
    wo = np.asarray(wo, dtype=np.float32)

    in_maps = []
    for core in range(NCORES):
        b, g = divmod(core, G)
        in_maps.append({
            "xT": _chunked(np.ascontiguousarray(hs[b].T), NE),
            "wq": _chunked(np.ascontiguousarray(wq[:, GD * g:GD * (g + 1)]), NE),
            "wk": _chunked(np.ascontiguousarray(wk[:, D * g:D * (g + 1)]), NE),
            "wv": _chunked(np.ascontiguousarray(wv[:, D * g:D * (g + 1)]), NE
                           ).astype(ml_dtypes.bfloat16),
            "wo": _chunked(np.ascontiguousarray(wo[GD * g:GD * (g + 1), :]), G),
            "cosT": cosT,
            "sinTf": sinTf,
        })

    res = run_bass_kernel_spmd(_NC, in_maps, list(range(NCORES)))
    LAST_RESULTS = res
    outs = [np.asarray(res.results[i]["out"], dtype=np.float32)
            for i in range(NCORES)]
    full = np.stack([sum(outs[b * G:(b + 1) * G]) for b in range(B)], axis=0)
    return full.astype(np.float32)


# revision 4
# speedup vs baseline: 1.1011x; 1.1011x over previous
"""GQA (16 q-heads / 4 kv-heads, D=128, S=2048, E=2048, B=2) on 8 trn2 cores.

Sharding: core = 4*b + g  (b in {0,1} batch, g in {0..3} kv-head group).
Each core computes its batch's 4 query heads (one kv group) end-to-end.

v4 design (v1 baseline 432us, v2 362us):
 - ALL matmul operands bf16 (x, wq/wk/wv/wo, qt/kt/at/vn/ot; numpy-verified
   rel err 0.59% vs 2e-2 budget). Input DMA bytes halved; every matmul runs
   at the full 1 cycle/row PE rate with no mixed-dtype modes.
 - Consolidated DMAs, host pre-permuted to [partition, chunk, free]; load
   order interleaves wk/x0 quarters then streams per-head wq so the PE is
   gated only ~4us at kernel start.
 - V projected directly into natural [s, d] layout (x-tile stationary x
   wv moving), no PE transposes.
 - Per-chunk qk tiles [D, 5, 512] (slot 0 = K, 1+h = Q head h): batched
   rotate-half partition swap (2 DMAs/chunk on the Act queue) and 3 wide
   [128,5,512] bf16 combines on DVE with stride-0 broadcast cos/sin APs.
   Per-chunk tiles keep phase B's first scores from waiting on chunk-3
   rope (dependency granularity).
 - Scores matmuls fill [128,2,512] 2-bank PSUM pairs; ONE wide exp (1024
   free) per pair into contiguous bf16 at[128,16,512].
 - Softmax denominator entirely off PE: wide bf16 tensor_tensor tree on
   DVE, gpsimd partition_all_reduce, DVE reciprocal, gpsimd mult.
 - o_proj(q-1) emitted mid-chunk (after attn(q,h1)) so its inputs are
   always normalized before the PE arrives; last chunk's PSUM drains via
   the idle Act engine; output rows DMA'd in halves as produced.
"""

import numpy as np
import ml_dtypes

import concourse.bass as bass
import concourse.bacc as bacc
import concourse.mybir as mybir
import concourse.tile as tile
from concourse import bass_isa
from concourse.ap import AP
from concourse.bass_utils import run_bass_kernel_spmd

B, S, E = 2, 2048, 2048
H, HKV, D = 16, 4, 128
G = H // HKV          # 4 query heads per kv group
GD = G * D            # 512 channels per group
NCORES = 8
SCALE = 1.0 / float(np.sqrt(D))
ROPE_BASE = 10000.0

NE = E // 128         # 16 e-chunks (contraction for projections)
NSC = S // 512        # 4 s-chunks of 512
NST = S // 128        # 16 s-tiles of 128

F32 = mybir.dt.float32
BF16 = mybir.dt.bfloat16
AF = mybir.ActivationFunctionType
OP = mybir.AluOpType


def _bcast_mid(ap2d, n):
    """[P, F] AP -> [P, n, F] AP with stride-0 middle dim (broadcast)."""
    dims = [list(x) for x in ap2d.ap]
    return AP(ap2d.tensor, ap2d.offset, [dims[0], [0, n], dims[1]])


def _emit(nc, tc, xT, wq, wk, wv, wo, cosT, sinTf, out):
    from contextlib import ExitStack
    es = ExitStack()
    with es:
        gpool = es.enter_context(tc.tile_pool(name="glob", bufs=1))
        # per s-chunk: slot 0 = K, slots 1..4 = Q heads ([d, s] layout)
        qk_c = [gpool.tile([D, 5, 512], BF16, tag=f"qk{q}", name=f"qk{q}")
                for q in range(NSC)]
        vn_sb = gpool.tile([128, NST, D], BF16, tag="vn")
        wo_sb = gpool.tile([128, G, E], BF16, tag="wo")

        # ================= phase A: projections + RoPE =================
        with (
            tc.tile_pool(name="phA", bufs=1) as pa,
            tc.tile_pool(name="xs", bufs=2) as xpool,
            tc.tile_pool(name="ropetmp", bufs=2) as rpool,
            tc.tile_pool(name="psA", bufs=1, space=bass.MemorySpace.PSUM) as psA,
        ):
            wk_sb = pa.tile([128, NE, D], BF16, tag="wk")
            cos_sb = pa.tile([D, S], BF16, tag="cos")
            sin_sb = pa.tile([D, S], BF16, tag="sin")
            wv_sb = pa.tile([128, NE, D], BF16, tag="wv")
            wq_sb = pa.tile([128, NE, GD], BF16, tag="wq")

            xsl = [xpool.tile([128, NE, 512], BF16, tag="xs", name=f"xs{q}")
                   for q in range(NSC)]
            # interleave wk and x0 quarters so K accumulation starts ASAP
            for qq in range(4):
                nc.sync.dma_start(out=wk_sb[:, 4 * qq:4 * qq + 4, :],
                                  in_=wk.ap()[:, 4 * qq:4 * qq + 4, :])
                nc.sync.dma_start(out=xsl[0][:, 4 * qq:4 * qq + 4, :],
                                  in_=xT.ap()[:, 4 * qq:4 * qq + 4, 0:512])
            nc.sync.dma_start(out=wv_sb[:], in_=wv.ap())
            for h in range(G):
                hd = slice(h * D, (h + 1) * D)
                nc.sync.dma_start(out=wq_sb[:, :, hd], in_=wq.ap()[:, :, hd])
            nc.sync.dma_start(out=cos_sb[:], in_=cosT.ap())
            nc.sync.dma_start(out=sin_sb[:], in_=sinTf.ap())
            nc.sync.dma_start(out=xsl[1][:], in_=xT.ap()[:, :, 512:1024])
            nc.sync.dma_start(out=xsl[2][:], in_=xT.ap()[:, :, 1024:1536])
            nc.sync.dma_start(out=xsl[3][:], in_=xT.ap()[:, :, 1536:2048])
            nc.sync.dma_start(out=wo_sb[:], in_=wo.ap())

            for q in range(NSC):
                sl = slice(q * 512, (q + 1) * 512)
                x = xsl[q]
                qraw = rpool.tile([128, 5, 512], BF16, tag="qraw")
                qswp = rpool.tile([128, 5, 512], BF16, tag="qswp")
                rot = rpool.tile([128, 5, 512], BF16, tag="rot")
                # K projection (transposed layout)
                ps = psA.tile([128, 512], F32, tag="proj", bufs=2)
                for j in range(NE):
                    nc.tensor.matmul(ps[:], wk_sb[:, j, :], x[:, j, :],
                                     start=(j == 0), stop=(j == NE - 1))
                nc.vector.tensor_copy(qraw[:, 0, :], ps[:])
                # V projection directly into natural [s, d] layout
                psv = psA.tile([128, 4, D], F32, tag="vproj", bufs=2)
                for st in range(4):
                    t = q * 4 + st
                    ssl128 = slice(st * 128, (st + 1) * 128)
                    for j in range(NE):
                        nc.tensor.matmul(psv[:, st, :], x[:, j, ssl128],
                                         wv_sb[:, j, :],
                                         start=(j == 0), stop=(j == NE - 1))
                    nc.vector.tensor_copy(vn_sb[:, t, :], psv[:, st, :])
                # Q projections
                for h in range(G):
                    ps = psA.tile([128, 512], F32, tag="proj", bufs=2)
                    for j in range(NE):
                        nc.tensor.matmul(ps[:], wq_sb[:, j, h * D:(h + 1) * D],
                                         x[:, j, :],
                                         start=(j == 0), stop=(j == NE - 1))
                    nc.vector.tensor_copy(qraw[:, 1 + h, :], ps[:])
                # batched rope for all 5 projections of this chunk:
                # partition swap via 2 DMAs (Act queue; Act idle in phase A)
                nc.scalar.dma_start(out=qswp[0:64, :, :], in_=qraw[64:128, :, :])
                nc.scalar.dma_start(out=qswp[64:128, :, :], in_=qraw[0:64, :, :])
                cb = _bcast_mid(cos_sb[:, sl], 5)
                sb = _bcast_mid(sin_sb[:, sl], 5)
                nc.vector.tensor_tensor(rot[:], qswp[:], sb, OP.mult)
                nc.vector.tensor_tensor(qraw[:], qraw[:], cb, OP.mult)
                nc.vector.tensor_tensor(qk_c[q][:], qraw[:], rot[:], OP.add)

        # ================= phase B+C: attention + o_proj interleaved ====
        with (
            tc.tile_pool(name="atp", bufs=3) as atpool,
            tc.tile_pool(name="otp", bufs=2) as otpool,
            tc.tile_pool(name="nrm", bufs=2) as nrmpool,
            tc.tile_pool(name="ost", bufs=2) as opool,
            tc.tile_pool(name="psB", bufs=1, space=bass.MemorySpace.PSUM) as psB,
        ):
            ot_tiles = {}

            def attn_iter(q, h):
                at = atpool.tile([128, NST, 512], BF16, tag="at")
                av = psB.tile([D, 512], F32, tag="av", bufs=2)
                for tg in range(8):
                    sc2 = psB.tile([128, 2, 512], F32, tag="sc", bufs=2)
                    for tt in range(2):
                        t = 2 * tg + tt
                        nc.tensor.matmul(sc2[:, tt, :],
                                         qk_c[t // 4][:, 0, (t % 4) * 128:
                                                      (t % 4 + 1) * 128],
                                         qk_c[q][:, 1 + h, :],
                                         start=True, stop=True)
                    nc.scalar.activation(at[:, 2 * tg:2 * tg + 2, :], sc2[:],
                                         AF.Exp, scale=SCALE)
                    for tt in range(2):
                        t = 2 * tg + tt
                        nc.tensor.matmul(av[:], vn_sb[:, t, :], at[:, t, :],
                                         start=(t == 0), stop=(t == NST - 1))
                ot = otpool.tile([D, 512], BF16, tag=f"ot{h}", name=f"ot{h}_{q}")
                ot_tiles[(q, h)] = ot
                with nc.allow_low_precision(reason="bf16 attention, verified 6e-3 rel err"):
                    nc.vector.tensor_copy(ot[:], av[:])
                    # denominator: wide bf16 pairwise tree on DVE (in-place),
                    # then cross-partition sum + broadcast on gpsimd
                    nc.vector.tensor_tensor(at[:, 0:8, :], at[:, 0:8, :],
                                            at[:, 8:16, :], OP.add)
                    nc.vector.tensor_tensor(at[:, 0:4, :], at[:, 0:4, :],
                                            at[:, 4:8, :], OP.add)
                    nc.vector.tensor_tensor(at[:, 0:2, :], at[:, 0:2, :],
                                            at[:, 2:4, :], OP.add)
                    acc = nrmpool.tile([128, 512], BF16, tag="acc")
                    nc.vector.tensor_tensor(acc[:], at[:, 0, :], at[:, 1, :],
                                            OP.add)
                    den = nrmpool.tile([128, 512], F32, tag="den")
                    nc.gpsimd.partition_all_reduce(den[:], acc[:], 128,
                                                   bass_isa.ReduceOp.add)
                    rc = nrmpool.tile([128, 512], BF16, tag="rc")
                    nc.vector.reciprocal(rc[:], den[:])
                    nc.gpsimd.tensor_tensor(ot[:], ot[:], rc[:], OP.mult)

            def oproj(q, last=False):
                for st in range(4):
                    s0 = q * 512 + st * 128
                    ostg = opool.tile([128, E], F32, tag="ostg")
                    for eo in range(4):
                        op_ps = psB.tile([128, 512], F32, tag="op", bufs=2)
                        for h in range(G):
                            nc.tensor.matmul(
                                op_ps[:],
                                ot_tiles[(q, h)][:, st * 128:(st + 1) * 128],
                                wo_sb[:, h, eo * 512:(eo + 1) * 512],
                                start=(h == 0), stop=(h == G - 1))
                        osl = slice(eo * 512, (eo + 1) * 512)
                        if last:
                            nc.scalar.copy(ostg[:, osl], op_ps[:])
                        else:
                            nc.vector.tensor_copy(ostg[:, osl], op_ps[:])
                        if eo == 1:
                            nc.sync.dma_start(out=out.ap()[s0:s0 + 128, 0:1024],
                                              in_=ostg[:, 0:1024])
                        elif eo == 3:
                            nc.sync.dma_start(out=out.ap()[s0:s0 + 128, 1024:2048],
                                              in_=ostg[:, 1024:2048])

            for q in range(NSC):
                for h in range(G):
                    attn_iter(q, h)
                    if h == 1 and q >= 1:
                        oproj(q - 1)
            oproj(NSC - 1, last=True)


def _build():
    nc = bacc.Bacc("TRN2", target_bir_lowering=False, debug=False,
                   num_devices=NCORES)
    xT = nc.dram_tensor("xT", [128, NE, S], BF16, kind="ExternalInput")
    wq = nc.dram_tensor("wq", [128, NE, GD], BF16, kind="ExternalInput")
    wk = nc.dram_tensor("wk", [128, NE, D], BF16, kind="ExternalInput")
    wv = nc.dram_tensor("wv", [128, NE, D], BF16, kind="ExternalInput")
    wo = nc.dram_tensor("wo", [128, G, E], BF16, kind="ExternalInput")
    cosT = nc.dram_tensor("cosT", [D, S], BF16, kind="ExternalInput")
    sinTf = nc.dram_tensor("sinTf", [D, S], BF16, kind="ExternalInput")
    out = nc.dram_tensor("out", [S, E], F32, kind="ExternalOutput")
    with tile.TileContext(nc) as tc:
        _emit(nc, tc, xT, wq, wk, wv, wo, cosT, sinTf, out)
    nc.compile()
    return nc


def _rope_tables():
    inv = 1.0 / (ROPE_BASE ** (np.arange(0, D, 2, dtype=np.float64) / D))
    t = np.arange(S, dtype=np.float64)
    freqs = t[:, None] * inv[None, :]                    # [S, D/2]
    emb = np.concatenate([freqs, freqs], axis=-1)        # [S, D]
    cosT = np.cos(emb).T.astype(np.float32)              # [D, S]
    sinT = np.sin(emb).T.astype(np.float32)
    sinTf = sinT.copy()
    sinTf[: D // 2] *= -1.0                              # fold rotate_half sign
    return (np.ascontiguousarray(cosT).astype(ml_dtypes.bfloat16),
            np.ascontiguousarray(sinTf).astype(ml_dtypes.bfloat16))


def _chunked(a, nchunk):
    """[E, F] -> [128, nchunk, F] bf16 with chunk c = rows c*128..(c+1)*128."""
    E_, F_ = a.shape
    return np.ascontiguousarray(
        a.reshape(nchunk, 128, F_).transpose(1, 0, 2)).astype(ml_dtypes.bfloat16)


_NC = None
LAST_RESULTS = None


def kernel(hidden_states, wq, wk, wv, wo):
    global _NC, LAST_RESULTS
    if _NC is None:
        _NC = _build()
    cosT, sinTf = _rope_tables()
    hs = np.asarray(hidden_states, dtype=np.float32)
    wq = np.asarray(wq, dtype=np.float32)
    wk = np.asarray(wk, dtype=np.float32)
    wv = np.asarray(wv, dtype=np.float32)
    wo = np.asarray(wo, dtype=np.float32)

    in_maps = []
    for core in range(NCORES):
        b, g = divmod(core, G)
        in_maps.append({
            "xT": _chunked(np.ascontiguousarray(hs[b].T), NE),
            "wq": _chunked(np.ascontiguousarray(wq[:, GD * g:GD * (g + 1)]), NE),
            "wk": _chunked(np.ascontiguousarray(wk[:, D * g:D * (g + 1)]), NE),
            "wv": _chunked(np.ascontiguousarray(wv[:, D * g:D * (g + 1)]), NE),
            "wo": _chunked(np.ascontiguousarray(wo[GD * g:GD * (g + 1), :]), G),
            "cosT": cosT,
            "sinTf": sinTf,
        })

    res = run_bass_kernel_spmd(_NC, in_maps, list(range(NCORES)))
    LAST_RESULTS = res
    outs = [np.asarray(res.results[i]["out"], dtype=np.float32)
            for i in range(NCORES)]
    full = np.stack([sum(outs[b * G:(b + 1) * G]) for b in range(B)], axis=0)
    return full.astype(np.float32)


# revision 6
# speedup vs baseline: 1.1754x; 1.0676x over previous
"""GQA (16 q-heads / 4 kv-heads, D=128, S=2048, E=2048, B=2) on 8 trn2 cores.

Sharding: core = 4*b + g  (b in {0,1} batch, g in {0..3} kv-head group).
Each core computes its batch's 4 query heads (one kv group) end-to-end.

v4 design (v1 baseline 432us, v2 362us):
 - ALL matmul operands bf16 (x, wq/wk/wv/wo, qt/kt/at/vn/ot; numpy-verified
   rel err 0.59% vs 2e-2 budget). Input DMA bytes halved; every matmul runs
   at the full 1 cycle/row PE rate with no mixed-dtype modes.
 - Consolidated DMAs, host pre-permuted to [partition, chunk, free]; load
   order interleaves wk/x0 quarters then streams per-head wq so the PE is
   gated only ~4us at kernel start.
 - V projected directly into natural [s, d] layout (x-tile stationary x
   wv moving), no PE transposes.
 - Per-chunk qk tiles [D, 5, 512] (slot 0 = K, 1+h = Q head h): batched
   rotate-half partition swap (2 DMAs/chunk on the Act queue) and 3 wide
   [128,5,512] bf16 combines on DVE with stride-0 broadcast cos/sin APs.
   Per-chunk tiles keep phase B's first scores from waiting on chunk-3
   rope (dependency granularity).
 - Scores matmuls fill [128,2,512] 2-bank PSUM pairs; ONE wide exp (1024
   free) per pair into contiguous bf16 at[128,16,512].
 - Softmax denominator entirely off PE: wide bf16 tensor_tensor tree on
   DVE, gpsimd partition_all_reduce, DVE reciprocal, gpsimd mult.
 - o_proj(q-1) emitted mid-chunk (after attn(q,h1)) so its inputs are
   always normalized before the PE arrives; last chunk's PSUM drains via
   the idle Act engine; output rows DMA'd in halves as produced.
"""

import numpy as np
import ml_dtypes

import concourse.bass as bass
import concourse.bacc as bacc
import concourse.mybir as mybir
import concourse.tile as tile
from concourse import bass_isa
from concourse.ap import AP
from concourse.bass_utils import run_bass_kernel_spmd

B, S, E = 2, 2048, 2048
H, HKV, D = 16, 4, 128
G = H // HKV          # 4 query heads per kv group
GD = G * D            # 512 channels per group
NCORES = 8
SCALE = 1.0 / float(np.sqrt(D))
ROPE_BASE = 10000.0

NE = E // 128         # 16 e-chunks (contraction for projections)
NSC = S // 512        # 4 s-chunks of 512
NST = S // 128        # 16 s-tiles of 128

F32 = mybir.dt.float32
BF16 = mybir.dt.bfloat16
AF = mybir.ActivationFunctionType
OP = mybir.AluOpType


def _bcast_mid(ap2d, n):
    """[P, F] AP -> [P, n, F] AP with stride-0 middle dim (broadcast)."""
    dims = [list(x) for x in ap2d.ap]
    return AP(ap2d.tensor, ap2d.offset, [dims[0], [0, n], dims[1]])


def _emit(nc, tc, xT, wq, wk, wv, wo, cosT, sinTf, out):
    from contextlib import ExitStack
    es = ExitStack()
    with es:
        gpool = es.enter_context(tc.tile_pool(name="glob", bufs=1))
        # per s-chunk: slot 0 = K, slots 1..4 = Q heads ([d, s] layout)
        qk_c = [gpool.tile([D, 5, 512], BF16, tag=f"qk{q}", name=f"qk{q}")
                for q in range(NSC)]
        vn_sb = gpool.tile([128, NST, D], BF16, tag="vn")
        wo_sb = gpool.tile([128, G, E], BF16, tag="wo")
        ones_sb = gpool.tile([128, 128], BF16, tag="ones")
        nc.vector.memset(ones_sb[:], 1.0)
        # cos/sin + rope scratch live in the global pool: putting them in a
        # phase-A pool would make that pool's closing drain wait on the
        # chunk-3 rope chain and stall phase B's first scores ~6us.
        cos_sb = gpool.tile([D, S], BF16, tag="cos")
        sin_sb = gpool.tile([D, S], BF16, tag="sin")
        rpool = es.enter_context(tc.tile_pool(name="ropetmp", bufs=2))

        # ================= phase A: projections + RoPE =================
        with (
            tc.tile_pool(name="phA", bufs=1) as pa,
            tc.tile_pool(name="xs", bufs=4) as xpool,
            tc.tile_pool(name="psA", bufs=1, space=bass.MemorySpace.PSUM) as psA,
        ):
            wk_sb = pa.tile([128, NE, D], BF16, tag="wk")
            wv_sb = pa.tile([128, NE, D], BF16, tag="wv")
            wq_sb = pa.tile([128, NE, GD], BF16, tag="wq")

            xsl = [xpool.tile([128, NE, 512], BF16, tag="xs", name=f"xs{q}")
                   for q in range(NSC)]
            # interleave wk and x0 quarters so K accumulation starts ASAP
            for qq in range(4):
                nc.sync.dma_start(out=wk_sb[:, 4 * qq:4 * qq + 4, :],
                                  in_=wk.ap()[:, 4 * qq:4 * qq + 4, :])
                nc.sync.dma_start(out=xsl[0][:, 4 * qq:4 * qq + 4, :],
                                  in_=xT.ap()[:, 4 * qq:4 * qq + 4, 0:512])
            nc.sync.dma_start(out=wv_sb[:], in_=wv.ap())
            for h in range(G):
                hd = slice(h * D, (h + 1) * D)
                nc.sync.dma_start(out=wq_sb[:, :, hd], in_=wq.ap()[:, :, hd])
            nc.sync.dma_start(out=cos_sb[:], in_=cosT.ap())
            nc.sync.dma_start(out=sin_sb[:], in_=sinTf.ap())
            nc.sync.dma_start(out=xsl[1][:], in_=xT.ap()[:, :, 512:1024])
            nc.sync.dma_start(out=xsl[2][:], in_=xT.ap()[:, :, 1024:1536])
            nc.sync.dma_start(out=xsl[3][:], in_=xT.ap()[:, :, 1536:2048])
            nc.sync.dma_start(out=wo_sb[:], in_=wo.ap())

            for q in range(NSC):
                sl = slice(q * 512, (q + 1) * 512)
                x = xsl[q]
                qraw = rpool.tile([128, 5, 512], BF16, tag="qraw")
                qswp = rpool.tile([128, 5, 512], BF16, tag="qswp")
                rot = rpool.tile([128, 5, 512], BF16, tag="rot")
                # K projection (transposed layout)
                ps = psA.tile([128, 512], F32, tag="proj", bufs=2)
                for j in range(NE):
                    nc.tensor.matmul(ps[:], wk_sb[:, j, :], x[:, j, :],
                                     start=(j == 0), stop=(j == NE - 1))
                nc.vector.tensor_copy(qraw[:, 0, :], ps[:])
                # V projection directly into natural [s, d] layout
                psv = psA.tile([128, 4, D], F32, tag="vproj", bufs=2)
                for st in range(4):
                    t = q * 4 + st
                    ssl128 = slice(st * 128, (st + 1) * 128)
                    for j in range(NE):
                        nc.tensor.matmul(psv[:, st, :], x[:, j, ssl128],
                                         wv_sb[:, j, :],
                                         start=(j == 0), stop=(j == NE - 1))
                    nc.vector.tensor_copy(vn_sb[:, t, :], psv[:, st, :])
                # Q projections
                for h in range(G):
                    ps = psA.tile([128, 512], F32, tag="proj", bufs=2)
                    for j in range(NE):
                        nc.tensor.matmul(ps[:], wq_sb[:, j, h * D:(h + 1) * D],
                                         x[:, j, :],
                                         start=(j == 0), stop=(j == NE - 1))
                    nc.vector.tensor_copy(qraw[:, 1 + h, :], ps[:])
                # batched rope for all 5 projections of this chunk:
                # partition swap via 2 DMAs (Act queue; Act idle in phase A)
                nc.scalar.dma_start(out=qswp[0:64, :, :], in_=qraw[64:128, :, :])
                nc.scalar.dma_start(out=qswp[64:128, :, :], in_=qraw[0:64, :, :])
                cb = _bcast_mid(cos_sb[:, sl], 5)
                sb = _bcast_mid(sin_sb[:, sl], 5)
                nc.vector.tensor_tensor(rot[:], qswp[:], sb, OP.mult)
                nc.vector.tensor_tensor(qraw[:], qraw[:], cb, OP.mult)
                nc.vector.tensor_tensor(qk_c[q][:], qraw[:], rot[:], OP.add)

        # ================= phase B+C: attention + o_proj interleaved ====
        with (
            tc.tile_pool(name="atp", bufs=3) as atpool,
            tc.tile_pool(name="otp", bufs=2) as otpool,
            tc.tile_pool(name="nrm", bufs=2) as nrmpool,
            tc.tile_pool(name="ost", bufs=2) as opool,
            tc.tile_pool(name="psB", bufs=1, space=bass.MemorySpace.PSUM) as psB,
        ):
            ot_tiles = {}

            def attn_iter(q, h, fast_norm=False):
                at = atpool.tile([128, NST, 512], BF16, tag="at")
                av = psB.tile([D, 512], F32, tag="av", bufs=2)
                if fast_norm:
                    sm = psB.tile([D, 512], F32, tag="av", bufs=2)
                for tg in range(8):
                    sc2 = psB.tile([128, 2, 512], F32, tag="sc", bufs=2)
                    for tt in range(2):
                        t = 2 * tg + tt
                        nc.tensor.matmul(sc2[:, tt, :],
                                         qk_c[t // 4][:, 0, (t % 4) * 128:
                                                      (t % 4 + 1) * 128],
                                         qk_c[q][:, 1 + h, :],
                                         start=True, stop=True)
                    nc.scalar.activation(at[:, 2 * tg:2 * tg + 2, :], sc2[:],
                                         AF.Exp, scale=SCALE)
                    for tt in range(2):
                        t = 2 * tg + tt
                        nc.tensor.matmul(av[:], vn_sb[:, t, :], at[:, t, :],
                                         start=(t == 0), stop=(t == NST - 1))
                        if fast_norm:
                            nc.tensor.matmul(sm[0:1, :], ones_sb[:, 0:1],
                                             at[:, t, :],
                                             start=(t == 0), stop=(t == NST - 1))
                ot = otpool.tile([D, 512], BF16, tag=f"ot{h}", name=f"ot{h}_{q}")
                ot_tiles[(q, h)] = ot
                with nc.allow_low_precision(reason="bf16 attention, verified 6e-3 rel err"):
                    nc.vector.tensor_copy(ot[:], av[:])
                    rc = nrmpool.tile([128, 512], BF16, tag="rc")
                    if fast_norm:
                        # last iteration: lowest-latency path so the final
                        # o_proj block is not left waiting on the DVE tree
                        rcr = nrmpool.tile([1, 512], BF16, tag="rcr")
                        nc.vector.reciprocal(rcr[:], sm[0:1, :])
                        bc = psB.tile([D, 512], F32, tag="av", bufs=2)
                        nc.tensor.matmul(bc[:], ones_sb[0:1, :], rcr[:],
                                         start=True, stop=True)
                        nc.vector.tensor_copy(rc[:], bc[:])
                    else:
                        # denominator off PE: wide bf16 tree on DVE
                        # (in-place), cross-partition sum on gpsimd
                        nc.vector.tensor_tensor(at[:, 0:8, :], at[:, 0:8, :],
                                                at[:, 8:16, :], OP.add)
                        nc.vector.tensor_tensor(at[:, 0:4, :], at[:, 0:4, :],
                                                at[:, 4:8, :], OP.add)
                        nc.vector.tensor_tensor(at[:, 0:2, :], at[:, 0:2, :],
                                                at[:, 2:4, :], OP.add)
                        acc = nrmpool.tile([128, 512], BF16, tag="acc")
                        nc.vector.tensor_tensor(acc[:], at[:, 0, :],
                                                at[:, 1, :], OP.add)
                        den = nrmpool.tile([128, 512], F32, tag="den")
                        nc.gpsimd.partition_all_reduce(den[:], acc[:], 128,
                                                       bass_isa.ReduceOp.add)
                        nc.vector.reciprocal(rc[:], den[:])
                    nc.gpsimd.tensor_tensor(ot[:], ot[:], rc[:], OP.mult)

            def oproj_st(q, st, last=False):
                s0 = q * 512 + st * 128
                ostg = opool.tile([128, E], F32, tag="ostg")
                for eo in range(4):
                    op_ps = psB.tile([128, 512], F32, tag="op", bufs=2)
                    for h in range(G):
                        nc.tensor.matmul(
                            op_ps[:],
                            ot_tiles[(q, h)][:, st * 128:(st + 1) * 128],
                            wo_sb[:, h, eo * 512:(eo + 1) * 512],
                            start=(h == 0), stop=(h == G - 1))
                    osl = slice(eo * 512, (eo + 1) * 512)
                    if last:
                        nc.scalar.copy(ostg[:, osl], op_ps[:])
                    else:
                        nc.vector.tensor_copy(ostg[:, osl], op_ps[:])
                    if eo == 1:
                        nc.sync.dma_start(out=out.ap()[s0:s0 + 128, 0:1024],
                                          in_=ostg[:, 0:1024])
                    elif eo == 3:
                        nc.sync.dma_start(out=out.ap()[s0:s0 + 128, 1024:2048],
                                          in_=ostg[:, 1024:2048])

            # o_proj st-blocks of chunk q-1 interleave with attn iterations
            # of chunk q so the Act engine (exp) never starves behind a
            # contiguous o_proj burst.
            for q in range(NSC):
                for h in range(G):
                    attn_iter(q, h, fast_norm=(q == NSC - 1 and h == G - 1))
                    if q >= 1:
                        oproj_st(q - 1, h)
            for st in range(4):
                oproj_st(NSC - 1, st, last=True)


def _build():
    nc = bacc.Bacc("TRN2", target_bir_lowering=False, debug=False,
                   num_devices=NCORES)
    xT = nc.dram_tensor("xT", [128, NE, S], BF16, kind="ExternalInput")
    wq = nc.dram_tensor("wq", [128, NE, GD], BF16, kind="ExternalInput")
    wk = nc.dram_tensor("wk", [128, NE, D], BF16, kind="ExternalInput")
    wv = nc.dram_tensor("wv", [128, NE, D], BF16, kind="ExternalInput")
    wo = nc.dram_tensor("wo", [128, G, E], BF16, kind="ExternalInput")
    cosT = nc.dram_tensor("cosT", [D, S], BF16, kind="ExternalInput")
    sinTf = nc.dram_tensor("sinTf", [D, S], BF16, kind="ExternalInput")
    out = nc.dram_tensor("out", [S, E], F32, kind="ExternalOutput")
    with tile.TileContext(nc) as tc:
        _emit(nc, tc, xT, wq, wk, wv, wo, cosT, sinTf, out)
    nc.compile()
    return nc


def _rope_tables():
    inv = 1.0 / (ROPE_BASE ** (np.arange(0, D, 2, dtype=np.float64) / D))
    t = np.arange(S, dtype=np.float64)
    freqs = t[:, None] * inv[None, :]                    # [S, D/2]
    emb = np.concatenate([freqs, freqs], axis=-1)        # [S, D]
    cosT = np.cos(emb).T.astype(np.float32)              # [D, S]
    sinT = np.sin(emb).T.astype(np.float32)
    sinTf = sinT.copy()
    sinTf[: D // 2] *= -1.0                              # fold rotate_half sign
    return (np.ascontiguousarray(cosT).astype(ml_dtypes.bfloat16),
            np.ascontiguousarray(sinTf).astype(ml_dtypes.bfloat16))


def _chunked(a, nchunk):
    """[E, F] -> [128, nchunk, F] bf16 with chunk c = rows c*128..(c+1)*128."""
    E_, F_ = a.shape
    return np.ascontiguousarray(
        a.reshape(nchunk, 128, F_).transpose(1, 0, 2)).astype(ml_dtypes.bfloat16)


_NC = None
LAST_RESULTS = None


def kernel(hidden_states, wq, wk, wv, wo):
    global _NC, LAST_RESULTS
    if _NC is None:
        _NC = _build()
    cosT, sinTf = _rope_tables()
    hs = np.asarray(hidden_states, dtype=np.float32)
    wq = np.asarray(wq, dtype=np.float32)
    wk = np.asarray(wk, dtype=np.float32)
    wv = np.asarray(wv, dtype=np.float32)
    wo = np.asarray(wo, dtype=np.float32)

    in_maps = []
    for core in range(NCORES):
        b, g = divmod(core, G)
        in_maps.append({
            "xT": _chunked(np.ascontiguousarray(hs[b].T), NE),
            "wq": _chunked(np.ascontiguousarray(wq[:, GD * g:GD * (g + 1)]), NE),
            "wk": _chunked(np.ascontiguousarray(wk[:, D * g:D * (g + 1)]), NE),
            "wv": _chunked(np.ascontiguousarray(wv[:, D * g:D * (g + 1)]), NE),
            "wo": _chunked(np.ascontiguousarray(wo[GD * g:GD * (g + 1), :]), G),
            "cosT": cosT,
            "sinTf": sinTf,
        })

    res = run_bass_kernel_spmd(_NC, in_maps, list(range(NCORES)))
    LAST_RESULTS = res
    outs = [np.asarray(res.results[i]["out"], dtype=np.float32)
            for i in range(NCORES)]
    full = np.stack([sum(outs[b * G:(b + 1) * G]) for b in range(B)], axis=0)
    return full.astype(np.float32)


# revision 8
# speedup vs baseline: 1.2120x; 1.0311x over previous
"""GQA (16 q-heads / 4 kv-heads, D=128, S=2048, E=2048, B=2) on 8 trn2 cores.

Sharding: core = 4*b + g  (b in {0,1} batch, g in {0..3} kv-head group).
Each core computes its batch's 4 query heads (one kv group) end-to-end.

v4 design (v1 baseline 432us, v2 362us):
 - ALL matmul operands bf16 (x, wq/wk/wv/wo, qt/kt/at/vn/ot; numpy-verified
   rel err 0.59% vs 2e-2 budget). Input DMA bytes halved; every matmul runs
   at the full 1 cycle/row PE rate with no mixed-dtype modes.
 - Consolidated DMAs, host pre-permuted to [partition, chunk, free]; load
   order interleaves wk/x0 quarters then streams per-head wq so the PE is
   gated only ~4us at kernel start.
 - V projected directly into natural [s, d] layout (x-tile stationary x
   wv moving), no PE transposes.
 - Per-chunk qk tiles [D, 5, 512] (slot 0 = K, 1+h = Q head h): batched
   rotate-half partition swap (2 DMAs/chunk on the Act queue) and 3 wide
   [128,5,512] bf16 combines on DVE with stride-0 broadcast cos/sin APs.
   Per-chunk tiles keep phase B's first scores from waiting on chunk-3
   rope (dependency granularity).
 - Scores matmuls fill [128,2,512] 2-bank PSUM pairs; ONE wide exp (1024
   free) per pair into contiguous bf16 at[128,16,512].
 - Softmax denominator entirely off PE: wide bf16 tensor_tensor tree on
   DVE, gpsimd partition_all_reduce, DVE reciprocal, gpsimd mult.
 - o_proj(q-1) emitted mid-chunk (after attn(q,h1)) so its inputs are
   always normalized before the PE arrives; last chunk's PSUM drains via
   the idle Act engine; output rows DMA'd in halves as produced.
"""

import numpy as np
import ml_dtypes

import concourse.bass as bass
import concourse.bacc as bacc
import concourse.mybir as mybir
import concourse.tile as tile
from concourse import bass_isa
from concourse.ap import AP
from concourse.bass_utils import run_bass_kernel_spmd

B, S, E = 2, 2048, 2048
H, HKV, D = 16, 4, 128
G = H // HKV          # 4 query heads per kv group
GD = G * D            # 512 channels per group
NCORES = 8
SCALE = 1.0 / float(np.sqrt(D))
ROPE_BASE = 10000.0

NE = E // 128         # 16 e-chunks (contraction for projections)
NSC = S // 512        # 4 s-chunks of 512
NST = S // 128        # 16 s-tiles of 128

F32 = mybir.dt.float32
BF16 = mybir.dt.bfloat16
AF = mybir.ActivationFunctionType
OP = mybir.AluOpType


def _bcast_mid(ap2d, n):
    """[P, F] AP -> [P, n, F] AP with stride-0 middle dim (broadcast)."""
    dims = [list(x) for x in ap2d.ap]
    return AP(ap2d.tensor, ap2d.offset, [dims[0], [0, n], dims[1]])


def _emit(nc, tc, xT, wq, wk, wv, wo, cosT, sinTf, out):
    from contextlib import ExitStack
    es = ExitStack()
    with es:
        gpool = es.enter_context(tc.tile_pool(name="glob", bufs=1))
        # per s-chunk: slot 0 = K, slots 1..4 = Q heads ([d, s] layout)
        qk_c = [gpool.tile([D, 5, 512], BF16, tag=f"qk{q}", name=f"qk{q}")
                for q in range(NSC)]
        vn_sb = gpool.tile([128, NST, D], BF16, tag="vn")
        wo_sb = gpool.tile([128, G, E], BF16, tag="wo")
        ones_sb = gpool.tile([128, 128], BF16, tag="ones")
        nc.vector.memset(ones_sb[:], 1.0)
        # cos/sin + rope scratch live in the global pool: putting them in a
        # phase-A pool would make that pool's closing drain wait on the
        # chunk-3 rope chain and stall phase B's first scores ~6us.
        cos_sb = gpool.tile([D, S], BF16, tag="cos")
        sin_sb = gpool.tile([D, S], BF16, tag="sin")
        rpool = es.enter_context(tc.tile_pool(name="ropetmp", bufs=2))

        # ================= phase A: projections + RoPE =================
        with (
            tc.tile_pool(name="phA", bufs=1) as pa,
            tc.tile_pool(name="xs", bufs=4) as xpool,
            tc.tile_pool(name="psA", bufs=1, space=bass.MemorySpace.PSUM) as psA,
        ):
            wk_sb = pa.tile([128, NE, D], BF16, tag="wk")
            wv_sb = pa.tile([128, NE, D], BF16, tag="wv")
            # per-head-major so each head's load is one 4KB-run DMA
            wq_sb = pa.tile([128, G, NE, D], BF16, tag="wq")

            xsl = [xpool.tile([128, NE, 512], BF16, tag="xs", name=f"xs{q}")
                   for q in range(NSC)]
            # interleave wk and x0 quarters so K accumulation starts ASAP
            for qq in range(4):
                nc.sync.dma_start(out=wk_sb[:, 4 * qq:4 * qq + 4, :],
                                  in_=wk.ap()[:, 4 * qq:4 * qq + 4, :])
                nc.sync.dma_start(out=xsl[0][:, 4 * qq:4 * qq + 4, :],
                                  in_=xT.ap()[:, 4 * qq:4 * qq + 4, 0:512])
            nc.sync.dma_start(out=wv_sb[:], in_=wv.ap())
            nc.sync.dma_start(out=wq_sb[:, 0], in_=wq.ap()[:, 0])
            nc.sync.dma_start(out=wq_sb[:, 1], in_=wq.ap()[:, 1])
            nc.sync.dma_start(out=xsl[1][:], in_=xT.ap()[:, :, 512:1024])
            nc.sync.dma_start(out=wq_sb[:, 2], in_=wq.ap()[:, 2])
            nc.sync.dma_start(out=wq_sb[:, 3], in_=wq.ap()[:, 3])
            nc.sync.dma_start(out=xsl[2][:], in_=xT.ap()[:, :, 1024:1536])
            nc.sync.dma_start(out=cos_sb[:], in_=cosT.ap())
            nc.sync.dma_start(out=sin_sb[:], in_=sinTf.ap())
            nc.sync.dma_start(out=xsl[3][:], in_=xT.ap()[:, :, 1536:2048])
            nc.sync.dma_start(out=wo_sb[:], in_=wo.ap())

            for q in range(NSC):
                sl = slice(q * 512, (q + 1) * 512)
                x = xsl[q]
                qraw = rpool.tile([128, 5, 512], BF16, tag="qraw")
                qswp = rpool.tile([128, 5, 512], BF16, tag="qswp")
                rot = rpool.tile([128, 5, 512], BF16, tag="rot")
                # K projection (transposed layout)
                ps = psA.tile([128, 512], F32, tag="proj", bufs=2)
                for j in range(NE):
                    nc.tensor.matmul(ps[:], wk_sb[:, j, :], x[:, j, :],
                                     start=(j == 0), stop=(j == NE - 1))
                nc.vector.tensor_copy(qraw[:, 0, :], ps[:])
                # V projection directly into natural [s, d] layout
                psv = psA.tile([128, 4, D], F32, tag="vproj", bufs=2)
                for st in range(4):
                    t = q * 4 + st
                    ssl128 = slice(st * 128, (st + 1) * 128)
                    for j in range(NE):
                        nc.tensor.matmul(psv[:, st, :], x[:, j, ssl128],
                                         wv_sb[:, j, :],
                                         start=(j == 0), stop=(j == NE - 1))
                    nc.vector.tensor_copy(vn_sb[:, t, :], psv[:, st, :])
                # Q projections
                for h in range(G):
                    ps = psA.tile([128, 512], F32, tag="proj", bufs=2)
                    for j in range(NE):
                        nc.tensor.matmul(ps[:], wq_sb[:, h, j, :],
                                         x[:, j, :],
                                         start=(j == 0), stop=(j == NE - 1))
                    nc.vector.tensor_copy(qraw[:, 1 + h, :], ps[:])
                # batched rope for all 5 projections of this chunk:
                # partition swap via 2 DMAs (Act queue; Act idle in phase A)
                nc.scalar.dma_start(out=qswp[0:64, :, :], in_=qraw[64:128, :, :])
                nc.scalar.dma_start(out=qswp[64:128, :, :], in_=qraw[0:64, :, :])
                cb = _bcast_mid(cos_sb[:, sl], 5)
                sb = _bcast_mid(sin_sb[:, sl], 5)
                nc.vector.tensor_tensor(rot[:], qswp[:], sb, OP.mult)
                nc.vector.tensor_tensor(qraw[:], qraw[:], cb, OP.mult)
                nc.vector.tensor_tensor(qk_c[q][:], qraw[:], rot[:], OP.add)

        # ================= phase B+C: attention + o_proj interleaved ====
        with (
            tc.tile_pool(name="atp", bufs=3) as atpool,
            tc.tile_pool(name="otp", bufs=2) as otpool,
            tc.tile_pool(name="nrm", bufs=2) as nrmpool,
            tc.tile_pool(name="ost", bufs=2) as opool,
            tc.tile_pool(name="psB", bufs=1, space=bass.MemorySpace.PSUM) as psB,
        ):
            ot_tiles = {}

            def attn_iter(q, h, fast_norm=False):
                at = atpool.tile([128, NST, 512], BF16, tag="at")
                av = psB.tile([D, 512], F32, tag="av", bufs=2)
                if fast_norm:
                    sm = psB.tile([D, 512], F32, tag="av", bufs=2)
                for tg in range(8):
                    sc2 = psB.tile([128, 2, 512], F32, tag="sc", bufs=2)
                    for tt in range(2):
                        t = 2 * tg + tt
                        nc.tensor.matmul(sc2[:, tt, :],
                                         qk_c[t // 4][:, 0, (t % 4) * 128:
                                                      (t % 4 + 1) * 128],
                                         qk_c[q][:, 1 + h, :],
                                         start=True, stop=True)
                    nc.scalar.activation(at[:, 2 * tg:2 * tg + 2, :], sc2[:],
                                         AF.Exp, scale=SCALE)
                    for tt in range(2):
                        t = 2 * tg + tt
                        nc.tensor.matmul(av[:], vn_sb[:, t, :], at[:, t, :],
                                         start=(t == 0), stop=(t == NST - 1))
                        if fast_norm:
                            nc.tensor.matmul(sm[0:1, :], ones_sb[:, 0:1],
                                             at[:, t, :],
                                             start=(t == 0), stop=(t == NST - 1))
                ot = otpool.tile([D, 512], BF16, tag=f"ot{h}", name=f"ot{h}_{q}")
                ot_tiles[(q, h)] = ot
                with nc.allow_low_precision(reason="bf16 attention, verified 6e-3 rel err"):
                    nc.vector.tensor_copy(ot[:], av[:])
                    rc = nrmpool.tile([128, 512], BF16, tag="rc")
                    if fast_norm:
                        # last iteration: lowest-latency path so the final
                        # o_proj block is not left waiting on the DVE tree
                        rcr = nrmpool.tile([1, 512], BF16, tag="rcr")
                        nc.vector.reciprocal(rcr[:], sm[0:1, :])
                        bc = psB.tile([D, 512], F32, tag="av", bufs=2)
                        nc.tensor.matmul(bc[:], ones_sb[0:1, :], rcr[:],
                                         start=True, stop=True)
                        nc.vector.tensor_copy(rc[:], bc[:])
                    else:
                        # denominator off PE: wide bf16 tree on DVE
                        # (in-place), cross-partition sum on gpsimd
                        nc.vector.tensor_tensor(at[:, 0:8, :], at[:, 0:8, :],
                                                at[:, 8:16, :], OP.add)
                        nc.vector.tensor_tensor(at[:, 0:4, :], at[:, 0:4, :],
                                                at[:, 4:8, :], OP.add)
                        nc.vector.tensor_tensor(at[:, 0:2, :], at[:, 0:2, :],
                                                at[:, 2:4, :], OP.add)
                        acc = nrmpool.tile([128, 512], BF16, tag="acc")
                        nc.vector.tensor_tensor(acc[:], at[:, 0, :],
                                                at[:, 1, :], OP.add)
                        den = nrmpool.tile([128, 512], F32, tag="den")
                        nc.gpsimd.partition_all_reduce(den[:], acc[:], 128,
                                                       bass_isa.ReduceOp.add)
                        nc.vector.reciprocal(rc[:], den[:])
                    nc.gpsimd.tensor_tensor(ot[:], ot[:], rc[:], OP.mult)

            def oproj_st(q, st, last=False):
                s0 = q * 512 + st * 128
                ostg = opool.tile([128, E], F32, tag="ostg")
                for eo in range(4):
                    op_ps = psB.tile([128, 512], F32, tag="op", bufs=2)
                    for h in range(G):
                        nc.tensor.matmul(
                            op_ps[:],
                            ot_tiles[(q, h)][:, st * 128:(st + 1) * 128],
                            wo_sb[:, h, eo * 512:(eo + 1) * 512],
                            start=(h == 0), stop=(h == G - 1))
                    osl = slice(eo * 512, (eo + 1) * 512)
                    if last:
                        nc.scalar.copy(ostg[:, osl], op_ps[:])
                    else:
                        nc.vector.tensor_copy(ostg[:, osl], op_ps[:])
                    if eo == 1:
                        nc.sync.dma_start(out=out.ap()[s0:s0 + 128, 0:1024],
                                          in_=ostg[:, 0:1024])
                    elif eo == 3:
                        nc.sync.dma_start(out=out.ap()[s0:s0 + 128, 1024:2048],
                                          in_=ostg[:, 1024:2048])

            # o_proj st-blocks of chunk q-1 interleave with attn iterations
            # of chunk q so the Act engine (exp) never starves behind a
            # contiguous o_proj burst.
            for q in range(NSC):
                for h in range(G):
                    attn_iter(q, h, fast_norm=(q == NSC - 1 and h == G - 1))
                    if q >= 1:
                        oproj_st(q - 1, h)
            for st in range(4):
                oproj_st(NSC - 1, st, last=True)


def _build():
    nc = bacc.Bacc("TRN2", target_bir_lowering=False, debug=False,
                   num_devices=NCORES)
    xT = nc.dram_tensor("xT", [128, NE, S], BF16, kind="ExternalInput")
    wq = nc.dram_tensor("wq", [128, G, NE, D], BF16, kind="ExternalInput")
    wk = nc.dram_tensor("wk", [128, NE, D], BF16, kind="ExternalInput")
    wv = nc.dram_tensor("wv", [128, NE, D], BF16, kind="ExternalInput")
    wo = nc.dram_tensor("wo", [128, G, E], BF16, kind="ExternalInput")
    cosT = nc.dram_tensor("cosT", [D, S], BF16, kind="ExternalInput")
    sinTf = nc.dram_tensor("sinTf", [D, S], BF16, kind="ExternalInput")
    out = nc.dram_tensor("out", [S, E], F32, kind="ExternalOutput")
    with tile.TileContext(nc) as tc:
        _emit(nc, tc, xT, wq, wk, wv, wo, cosT, sinTf, out)
    nc.compile()
    return nc


def _rope_tables():
    inv = 1.0 / (ROPE_BASE ** (np.arange(0, D, 2, dtype=np.float64) / D))
    t = np.arange(S, dtype=np.float64)
    freqs = t[:, None] * inv[None, :]                    # [S, D/2]
    emb = np.concatenate([freqs, freqs], axis=-1)        # [S, D]
    cosT = np.cos(emb).T.astype(np.float32)              # [D, S]
    sinT = np.sin(emb).T.astype(np.float32)
    sinTf = sinT.copy()
    sinTf[: D // 2] *= -1.0                              # fold rotate_half sign
    return (np.ascontiguousarray(cosT).astype(ml_dtypes.bfloat16),
            np.ascontiguousarray(sinTf).astype(ml_dtypes.bfloat16))


def _chunked(a, nchunk):
    """[E, F] -> [128, nchunk, F] bf16 with chunk c = rows c*128..(c+1)*128."""
    E_, F_ = a.shape
    return np.ascontiguousarray(
        a.reshape(nchunk, 128, F_).transpose(1, 0, 2)).astype(ml_dtypes.bfloat16)


def _wq_chunked(a):
    """[E, GD] -> [128, G, NE, D] bf16, head-major with 4KB runs."""
    return np.ascontiguousarray(
        a.reshape(NE, 128, G, D).transpose(1, 2, 0, 3)).astype(ml_dtypes.bfloat16)


_NC = None
LAST_RESULTS = None


def kernel(hidden_states, wq, wk, wv, wo):
    global _NC, LAST_RESULTS
    if _NC is None:
        _NC = _build()
    cosT, sinTf = _rope_tables()
    hs = np.asarray(hidden_states, dtype=np.float32)
    wq = np.asarray(wq, dtype=np.float32)
    wk = np.asarray(wk, dtype=np.float32)
    wv = np.asarray(wv, dtype=np.float32)
    wo = np.asarray(wo, dtype=np.float32)

    in_maps = []
    for core in range(NCORES):
        b, g = divmod(core, G)
        in_maps.append({
            "xT": _chunked(np.ascontiguousarray(hs[b].T), NE),
            "wq": _wq_chunked(wq[:, GD * g:GD * (g + 1)]),
            "wk": _chunked(np.ascontiguousarray(wk[:, D * g:D * (g + 1)]), NE),
            "wv": _chunked(np.ascontiguousarray(wv[:, D * g:D * (g + 1)]), NE),
            "wo": _chunked(np.ascontiguousarray(wo[GD * g:GD * (g + 1), :]), G),
            "cosT": cosT,
            "sinTf": sinTf,
        })

    res = run_bass_kernel_spmd(_NC, in_maps, list(range(NCORES)))
    LAST_RESULTS = res
    outs = [np.asarray(res.results[i]["out"], dtype=np.float32)
            for i in range(NCORES)]
    full = np.stack([sum(outs[b * G:(b + 1) * G]) for b in range(B)], axis=0)
    return full.astype(np.float32)


# revision 10
# speedup vs baseline: 1.2835x; 1.0590x over previous
"""GQA (16 q-heads / 4 kv-heads, D=128, S=2048, E=2048, B=2) on 8 trn2 cores.

Sharding: core = 4*b + g  (b in {0,1} batch, g in {0..3} kv-head group).
Each core computes its batch's 4 query heads (one kv group) end-to-end.

v4 design (v1 baseline 432us, v2 362us):
 - ALL matmul operands bf16 (x, wq/wk/wv/wo, qt/kt/at/vn/ot; numpy-verified
   rel err 0.59% vs 2e-2 budget). Input DMA bytes halved; every matmul runs
   at the full 1 cycle/row PE rate with no mixed-dtype modes.
 - Consolidated DMAs, host pre-permuted to [partition, chunk, free]; load
   order interleaves wk/x0 quarters then streams per-head wq so the PE is
   gated only ~4us at kernel start.
 - V projected directly into natural [s, d] layout (x-tile stationary x
   wv moving), no PE transposes.
 - Per-chunk qk tiles [D, 5, 512] (slot 0 = K, 1+h = Q head h): batched
   rotate-half partition swap (2 DMAs/chunk on the Act queue) and 3 wide
   [128,5,512] bf16 combines on DVE with stride-0 broadcast cos/sin APs.
   Per-chunk tiles keep phase B's first scores from waiting on chunk-3
   rope (dependency granularity).
 - Scores matmuls fill [128,2,512] 2-bank PSUM pairs; ONE wide exp (1024
   free) per pair into contiguous bf16 at[128,16,512].
 - Softmax denominator entirely off PE: wide bf16 tensor_tensor tree on
   DVE, gpsimd partition_all_reduce, DVE reciprocal, gpsimd mult.
 - o_proj(q-1) emitted mid-chunk (after attn(q,h1)) so its inputs are
   always normalized before the PE arrives; last chunk's PSUM drains via
   the idle Act engine; output rows DMA'd in halves as produced.
"""

import numpy as np
import ml_dtypes

import concourse.bass as bass
import concourse.bacc as bacc
import concourse.mybir as mybir
import concourse.tile as tile
from concourse import bass_isa
from concourse.ap import AP
from concourse.bass_utils import run_bass_kernel_spmd

B, S, E = 2, 2048, 2048
H, HKV, D = 16, 4, 128
G = H // HKV          # 4 query heads per kv group
GD = G * D            # 512 channels per group
NCORES = 8
SCALE = 1.0 / float(np.sqrt(D))
ROPE_BASE = 10000.0

NE = E // 128         # 16 e-chunks (contraction for projections)
NSC = S // 512        # 4 s-chunks of 512
NST = S // 128        # 16 s-tiles of 128

F32 = mybir.dt.float32
BF16 = mybir.dt.bfloat16
AF = mybir.ActivationFunctionType
OP = mybir.AluOpType


def _bcast_mid(ap2d, n):
    """[P, F] AP -> [P, n, F] AP with stride-0 middle dim (broadcast)."""
    dims = [list(x) for x in ap2d.ap]
    return AP(ap2d.tensor, ap2d.offset, [dims[0], [0, n], dims[1]])


def _emit(nc, tc, xT, wq, wk, wv, wo, cosT, sinTf, out):
    from contextlib import ExitStack
    es = ExitStack()
    with es:
        gpool = es.enter_context(tc.tile_pool(name="glob", bufs=1))
        # per s-chunk: slot 0 = K, slots 1..4 = Q heads ([d, s] layout)
        qk_c = [gpool.tile([D, 5, 512], BF16, tag=f"qk{q}", name=f"qk{q}")
                for q in range(NSC)]
        vn_sb = gpool.tile([128, NST, D], BF16, tag="vn")
        wo_sb = gpool.tile([128, G, E], BF16, tag="wo")
        ones_sb = gpool.tile([128, 128], BF16, tag="ones")
        nc.vector.memset(ones_sb[:], 1.0)
        # cos/sin + rope scratch live in the global pool: putting them in a
        # phase-A pool would make that pool's closing drain wait on the
        # chunk-3 rope chain and stall phase B's first scores ~6us.
        cos_sb = gpool.tile([D, S], BF16, tag="cos")
        sin_sb = gpool.tile([D, S], BF16, tag="sin")
        rpool = es.enter_context(tc.tile_pool(name="ropetmp", bufs=2))

        # ================= phase A: projections + RoPE =================
        with (
            tc.tile_pool(name="phA", bufs=1) as pa,
            tc.tile_pool(name="xs", bufs=4) as xpool,
            tc.tile_pool(name="psA", bufs=1, space=bass.MemorySpace.PSUM) as psA,
        ):
            wk_sb = pa.tile([128, NE, D], BF16, tag="wk")
            wv_sb = pa.tile([128, NE, D], BF16, tag="wv")
            # per-head-major so each head's load is one 4KB-run DMA
            wq_sb = pa.tile([128, G, NE, D], BF16, tag="wq")

            xsl = [xpool.tile([128, NE, 512], BF16, tag="xs", name=f"xs{q}")
                   for q in range(NSC)]
            # interleave wk and x0 quarters so K accumulation starts ASAP
            for qq in range(4):
                nc.sync.dma_start(out=wk_sb[:, 4 * qq:4 * qq + 4, :],
                                  in_=wk.ap()[:, 4 * qq:4 * qq + 4, :])
                nc.sync.dma_start(out=xsl[0][:, 4 * qq:4 * qq + 4, :],
                                  in_=xT.ap()[:, 4 * qq:4 * qq + 4, 0:512])
            nc.sync.dma_start(out=wv_sb[:], in_=wv.ap())
            nc.sync.dma_start(out=wq_sb[:, 0], in_=wq.ap()[:, 0])
            nc.sync.dma_start(out=wq_sb[:, 1], in_=wq.ap()[:, 1])
            nc.sync.dma_start(out=xsl[1][:], in_=xT.ap()[:, :, 512:1024])
            nc.sync.dma_start(out=wq_sb[:, 2], in_=wq.ap()[:, 2])
            nc.sync.dma_start(out=wq_sb[:, 3], in_=wq.ap()[:, 3])
            nc.sync.dma_start(out=xsl[2][:], in_=xT.ap()[:, :, 1024:1536])
            nc.sync.dma_start(out=cos_sb[:], in_=cosT.ap())
            nc.sync.dma_start(out=sin_sb[:], in_=sinTf.ap())
            nc.sync.dma_start(out=xsl[3][:], in_=xT.ap()[:, :, 1536:2048])
            nc.sync.dma_start(out=wo_sb[:], in_=wo.ap())

            for q in range(NSC):
                sl = slice(q * 512, (q + 1) * 512)
                x = xsl[q]
                qraw = rpool.tile([128, 5, 512], BF16, tag="qraw")
                rot = rpool.tile([128, 5, 512], BF16, tag="rot")
                # K projection (transposed layout)
                ps = psA.tile([128, 512], F32, tag="proj", bufs=2)
                for j in range(NE):
                    nc.tensor.matmul(ps[:], wk_sb[:, j, :], x[:, j, :],
                                     start=(j == 0), stop=(j == NE - 1))
                nc.vector.tensor_copy(qraw[:, 0, :], ps[:])
                # V projection directly into natural [s, d] layout
                psv = psA.tile([128, 4, D], F32, tag="vproj", bufs=2)
                for st in range(4):
                    t = q * 4 + st
                    ssl128 = slice(st * 128, (st + 1) * 128)
                    for j in range(NE):
                        nc.tensor.matmul(psv[:, st, :], x[:, j, ssl128],
                                         wv_sb[:, j, :],
                                         start=(j == 0), stop=(j == NE - 1))
                    nc.vector.tensor_copy(vn_sb[:, t, :], psv[:, st, :])
                # Q projections
                for h in range(G):
                    ps = psA.tile([128, 512], F32, tag="proj", bufs=2)
                    for j in range(NE):
                        nc.tensor.matmul(ps[:], wq_sb[:, h, j, :],
                                         x[:, j, :],
                                         start=(j == 0), stop=(j == NE - 1))
                    nc.vector.tensor_copy(qraw[:, 1 + h, :], ps[:])
                # batched rope for all 5 projections of this chunk.
                # rotate_half's partition swap is done by reading the OTHER
                # half's partitions directly (cross-base-partition operands),
                # so no SBUF->SBUF DMA competes with the big input loads.
                cb = _bcast_mid(cos_sb[:, sl], 5)
                nc.vector.tensor_tensor(rot[0:64, :, :], qraw[64:128, :, :],
                                        _bcast_mid(sin_sb[0:64, sl], 5),
                                        OP.mult)
                nc.vector.tensor_tensor(rot[64:128, :, :], qraw[0:64, :, :],
                                        _bcast_mid(sin_sb[64:128, sl], 5),
                                        OP.mult)
                nc.vector.tensor_tensor(qraw[:], qraw[:], cb, OP.mult)
                nc.vector.tensor_tensor(qk_c[q][:], qraw[:], rot[:], OP.add)

        # ================= phase B+C: attention + o_proj interleaved ====
        with (
            tc.tile_pool(name="atp", bufs=3) as atpool,
            tc.tile_pool(name="otp", bufs=2) as otpool,
            tc.tile_pool(name="nrm", bufs=2) as nrmpool,
            tc.tile_pool(name="ost", bufs=2) as opool,
            tc.tile_pool(name="psB", bufs=1, space=bass.MemorySpace.PSUM) as psB,
        ):
            ot_tiles = {}

            def attn_iter(q, h, fast_norm=False):
                at = atpool.tile([128, NST, 512], BF16, tag="at")
                av = psB.tile([D, 512], F32, tag="av", bufs=2)
                if fast_norm:
                    sm = psB.tile([D, 512], F32, tag="av", bufs=2)
                for tg in range(8):
                    sc2 = psB.tile([128, 2, 512], F32, tag="sc", bufs=2)
                    for tt in range(2):
                        t = 2 * tg + tt
                        nc.tensor.matmul(sc2[:, tt, :],
                                         qk_c[t // 4][:, 0, (t % 4) * 128:
                                                      (t % 4 + 1) * 128],
                                         qk_c[q][:, 1 + h, :],
                                         start=True, stop=True)
                    nc.scalar.activation(at[:, 2 * tg:2 * tg + 2, :], sc2[:],
                                         AF.Exp, scale=SCALE)
                    for tt in range(2):
                        t = 2 * tg + tt
                        nc.tensor.matmul(av[:], vn_sb[:, t, :], at[:, t, :],
                                         start=(t == 0), stop=(t == NST - 1))
                        if fast_norm:
                            nc.tensor.matmul(sm[0:1, :], ones_sb[:, 0:1],
                                             at[:, t, :],
                                             start=(t == 0), stop=(t == NST - 1))
                ot = otpool.tile([D, 512], BF16, tag=f"ot{h}", name=f"ot{h}_{q}")
                ot_tiles[(q, h)] = ot
                with nc.allow_low_precision(reason="bf16 attention, verified 6e-3 rel err"):
                    nc.vector.tensor_copy(ot[:], av[:])
                    rc = nrmpool.tile([128, 512], BF16, tag="rc")
                    if fast_norm:
                        # last iteration: lowest-latency path so the final
                        # o_proj block is not left waiting on the DVE tree
                        rcr = nrmpool.tile([1, 512], BF16, tag="rcr")
                        nc.vector.reciprocal(rcr[:], sm[0:1, :])
                        bc = psB.tile([D, 512], F32, tag="av", bufs=2)
                        nc.tensor.matmul(bc[:], ones_sb[0:1, :], rcr[:],
                                         start=True, stop=True)
                        nc.vector.tensor_copy(rc[:], bc[:])
                    else:
                        # denominator off PE: wide bf16 tree on DVE
                        # (in-place), cross-partition sum on gpsimd
                        nc.vector.tensor_tensor(at[:, 0:8, :], at[:, 0:8, :],
                                                at[:, 8:16, :], OP.add)
                        nc.vector.tensor_tensor(at[:, 0:4, :], at[:, 0:4, :],
                                                at[:, 4:8, :], OP.add)
                        nc.vector.tensor_tensor(at[:, 0:2, :], at[:, 0:2, :],
                                                at[:, 2:4, :], OP.add)
                        acc = nrmpool.tile([128, 512], BF16, tag="acc")
                        nc.vector.tensor_tensor(acc[:], at[:, 0, :],
                                                at[:, 1, :], OP.add)
                        den = nrmpool.tile([128, 512], F32, tag="den")
                        nc.gpsimd.partition_all_reduce(den[:], acc[:], 128,
                                                       bass_isa.ReduceOp.add)
                        nc.vector.reciprocal(rc[:], den[:])
                    nc.gpsimd.tensor_tensor(ot[:], ot[:], rc[:], OP.mult)

            def oproj_st(q, st, last=False):
                s0 = q * 512 + st * 128
                ostg = opool.tile([128, E], F32, tag="ostg")
                for eo in range(4):
                    op_ps = psB.tile([128, 512], F32, tag="op", bufs=2)
                    for h in range(G):
                        nc.tensor.matmul(
                            op_ps[:],
                            ot_tiles[(q, h)][:, st * 128:(st + 1) * 128],
                            wo_sb[:, h, eo * 512:(eo + 1) * 512],
                            start=(h == 0), stop=(h == G - 1))
                    osl = slice(eo * 512, (eo + 1) * 512)
                    if last:
                        nc.scalar.copy(ostg[:, osl], op_ps[:])
                    else:
                        nc.vector.tensor_copy(ostg[:, osl], op_ps[:])
                    if eo == 1:
                        nc.sync.dma_start(out=out.ap()[s0:s0 + 128, 0:1024],
                                          in_=ostg[:, 0:1024])
                    elif eo == 3:
                        nc.sync.dma_start(out=out.ap()[s0:s0 + 128, 1024:2048],
                                          in_=ostg[:, 1024:2048])

            # o_proj st-blocks of chunk q-1 interleave with attn iterations
            # of chunk q so the Act engine (exp) never starves behind a
            # contiguous o_proj burst.
            for q in range(NSC):
                for h in range(G):
                    attn_iter(q, h, fast_norm=(q == NSC - 1 and h == G - 1))
                    if q >= 1:
                        oproj_st(q - 1, h)
            for st in range(4):
                oproj_st(NSC - 1, st, last=True)


def _build():
    nc = bacc.Bacc("TRN2", target_bir_lowering=False, debug=False,
                   num_devices=NCORES)
    xT = nc.dram_tensor("xT", [128, NE, S], BF16, kind="ExternalInput")
    wq = nc.dram_tensor("wq", [128, G, NE, D], BF16, kind="ExternalInput")
    wk = nc.dram_tensor("wk", [128, NE, D], BF16, kind="ExternalInput")
    wv = nc.dram_tensor("wv", [128, NE, D], BF16, kind="ExternalInput")
    wo = nc.dram_tensor("wo", [128, G, E], BF16, kind="ExternalInput")
    cosT = nc.dram_tensor("cosT", [D, S], BF16, kind="ExternalInput")
    sinTf = nc.dram_tensor("sinTf", [D, S], BF16, kind="ExternalInput")
    out = nc.dram_tensor("out", [S, E], F32, kind="ExternalOutput")
    with tile.TileContext(nc) as tc:
        _emit(nc, tc, xT, wq, wk, wv, wo, cosT, sinTf, out)
    nc.compile()
    return nc


def _rope_tables():
    inv = 1.0 / (ROPE_BASE ** (np.arange(0, D, 2, dtype=np.float64) / D))
    t = np.arange(S, dtype=np.float64)
    freqs = t[:, None] * inv[None, :]                    # [S, D/2]
    emb = np.concatenate([freqs, freqs], axis=-1)        # [S, D]
    cosT = np.cos(emb).T.astype(np.float32)              # [D, S]
    sinT = np.sin(emb).T.astype(np.float32)
    sinTf = sinT.copy()
    sinTf[: D // 2] *= -1.0                              # fold rotate_half sign
    return (np.ascontiguousarray(cosT).astype(ml_dtypes.bfloat16),
            np.ascontiguousarray(sinTf).astype(ml_dtypes.bfloat16))


def _chunked(a, nchunk):
    """[E, F] -> [128, nchunk, F] bf16 with chunk c = rows c*128..(c+1)*128."""
    E_, F_ = a.shape
    return np.ascontiguousarray(
        a.reshape(nchunk, 128, F_).transpose(1, 0, 2)).astype(ml_dtypes.bfloat16)


def _wq_chunked(a):
    """[E, GD] -> [128, G, NE, D] bf16, head-major with 4KB runs."""
    return np.ascontiguousarray(
        a.reshape(NE, 128, G, D).transpose(1, 2, 0, 3)).astype(ml_dtypes.bfloat16)


_NC = None
LAST_RESULTS = None


def kernel(hidden_states, wq, wk, wv, wo):
    global _NC, LAST_RESULTS
    if _NC is None:
        _NC = _build()
    cosT, sinTf = _rope_tables()
    hs = np.asarray(hidden_states, dtype=np.float32)
    wq = np.asarray(wq, dtype=np.float32)
    wk = np.asarray(wk, dtype=np.float32)
    wv = np.asarray(wv, dtype=np.float32)
    wo = np.asarray(wo, dtype=np.float32)

    in_maps = []
    for core in range(NCORES):
        b, g = divmod(core, G)
        in_maps.append({
            "xT": _chunked(np.ascontiguousarray(hs[b].T), NE),
            "wq": _wq_chunked(wq[:, GD * g:GD * (g + 1)]),
            "wk": _chunked(np.ascontiguousarray(wk[:, D * g:D * (g + 1)]), NE),
            "wv": _chunked(np.ascontiguousarray(wv[:, D * g:D * (g + 1)]), NE),
            "wo": _chunked(np.ascontiguousarray(wo[GD * g:GD * (g + 1), :]), G),
            "cosT": cosT,
            "sinTf": sinTf,
        })

    res = run_bass_kernel_spmd(_NC, in_maps, list(range(NCORES)))
    LAST_RESULTS = res
    outs = [np.asarray(res.results[i]["out"], dtype=np.float32)
            for i in range(NCORES)]
    full = np.stack([sum(outs[b * G:(b + 1) * G]) for b in range(B)], axis=0)
    return full.astype(np.float32)


# revision 14
# speedup vs baseline: 1.3057x; 1.0172x over previous
"""GQA (16 q-heads / 4 kv-heads, D=128, S=2048, E=2048, B=2) on 8 trn2 cores.

Sharding: core = 4*b + g  (b in {0,1} batch, g in {0..3} kv-head group).
Each core computes its batch's 4 query heads (one kv group) end-to-end.

v4 design (v1 baseline 432us, v2 362us):
 - ALL matmul operands bf16 (x, wq/wk/wv/wo, qt/kt/at/vn/ot; numpy-verified
   rel err 0.59% vs 2e-2 budget). Input DMA bytes halved; every matmul runs
   at the full 1 cycle/row PE rate with no mixed-dtype modes.
 - Consolidated DMAs, host pre-permuted to [partition, chunk, free]; load
   order interleaves wk/x0 quarters then streams per-head wq so the PE is
   gated only ~4us at kernel start.
 - V projected directly into natural [s, d] layout (x-tile stationary x
   wv moving), no PE transposes.
 - Per-chunk qk tiles [D, 5, 512] (slot 0 = K, 1+h = Q head h): batched
   rotate-half partition swap (2 DMAs/chunk on the Act queue) and 3 wide
   [128,5,512] bf16 combines on DVE with stride-0 broadcast cos/sin APs.
   Per-chunk tiles keep phase B's first scores from waiting on chunk-3
   rope (dependency granularity).
 - Scores matmuls fill [128,2,512] 2-bank PSUM pairs; ONE wide exp (1024
   free) per pair into contiguous bf16 at[128,16,512].
 - Softmax denominator entirely off PE: wide bf16 tensor_tensor tree on
   DVE, gpsimd partition_all_reduce, DVE reciprocal, gpsimd mult.
 - o_proj(q-1) emitted mid-chunk (after attn(q,h1)) so its inputs are
   always normalized before the PE arrives; last chunk's PSUM drains via
   the idle Act engine; output rows DMA'd in halves as produced.
"""

import numpy as np
import ml_dtypes

import concourse.bass as bass
import concourse.bacc as bacc
import concourse.mybir as mybir
import concourse.tile as tile
from concourse import bass_isa
from concourse.ap import AP
from concourse.bass_utils import run_bass_kernel_spmd

B, S, E = 2, 2048, 2048
H, HKV, D = 16, 4, 128
G = H // HKV          # 4 query heads per kv group
GD = G * D            # 512 channels per group
NCORES = 8
SCALE = 1.0 / float(np.sqrt(D))
ROPE_BASE = 10000.0

NE = E // 128         # 16 e-chunks (contraction for projections)
NSC = S // 512        # 4 s-chunks of 512
NST = S // 128        # 16 s-tiles of 128

F32 = mybir.dt.float32
BF16 = mybir.dt.bfloat16
AF = mybir.ActivationFunctionType
OP = mybir.AluOpType


def _bcast_mid(ap2d, n):
    """[P, F] AP -> [P, n, F] AP with stride-0 middle dim (broadcast)."""
    dims = [list(x) for x in ap2d.ap]
    return AP(ap2d.tensor, ap2d.offset, [dims[0], [0, n], dims[1]])


def _emit(nc, tc, xT, wq, wk, wv, wo, cosT, sinTf, out):
    from contextlib import ExitStack
    es = ExitStack()
    with es:
        gpool = es.enter_context(tc.tile_pool(name="glob", bufs=1))
        # per s-chunk: slot 0 = K, slots 1..4 = Q heads ([d, s] layout)
        qk_c = [gpool.tile([D, 5, 512], BF16, tag=f"qk{q}", name=f"qk{q}")
                for q in range(NSC)]
        vn_sb = gpool.tile([128, NST, D], BF16, tag="vn")
        wo_sb = gpool.tile([128, G, E], BF16, tag="wo")
        ones_sb = gpool.tile([128, 128], BF16, tag="ones")
        nc.vector.memset(ones_sb[:], 1.0)
        # cos/sin + rope scratch live in the global pool: putting them in a
        # phase-A pool would make that pool's closing drain wait on the
        # chunk-3 rope chain and stall phase B's first scores ~6us.
        cos_sb = gpool.tile([D, S], BF16, tag="cos")
        sin_sb = gpool.tile([D, S], BF16, tag="sin")
        rpool = es.enter_context(tc.tile_pool(name="ropetmp", bufs=2))

        # ================= phase A: projections + RoPE =================
        with (
            tc.tile_pool(name="phA", bufs=1) as pa,
            tc.tile_pool(name="xs", bufs=4) as xpool,
            tc.tile_pool(name="psA", bufs=1, space=bass.MemorySpace.PSUM) as psA,
        ):
            wk_sb = pa.tile([128, NE, D], BF16, tag="wk")
            wv_sb = pa.tile([128, NE, D], BF16, tag="wv")
            # per-head-major so each head's load is one 4KB-run DMA
            wq_sb = pa.tile([128, G, NE, D], BF16, tag="wq")

            xsl = [xpool.tile([128, NE, 512], BF16, tag="xs", name=f"xs{q}")
                   for q in range(NSC)]
            # interleave wk and x0 pieces so K accumulation starts ASAP;
            # the first pair is halved again to cut the cold-start wait
            nc.sync.dma_start(out=wk_sb[:, 0:2, :], in_=wk.ap()[:, 0:2, :])
            nc.sync.dma_start(out=xsl[0][:, 0:2, :],
                              in_=xT.ap()[:, 0:2, 0:512])
            nc.sync.dma_start(out=wk_sb[:, 2:4, :], in_=wk.ap()[:, 2:4, :])
            nc.sync.dma_start(out=xsl[0][:, 2:4, :],
                              in_=xT.ap()[:, 2:4, 0:512])
            for qq in range(1, 4):
                nc.sync.dma_start(out=wk_sb[:, 4 * qq:4 * qq + 4, :],
                                  in_=wk.ap()[:, 4 * qq:4 * qq + 4, :])
                nc.sync.dma_start(out=xsl[0][:, 4 * qq:4 * qq + 4, :],
                                  in_=xT.ap()[:, 4 * qq:4 * qq + 4, 0:512])
            nc.sync.dma_start(out=wv_sb[:], in_=wv.ap())
            nc.sync.dma_start(out=wq_sb[:, 0], in_=wq.ap()[:, 0])
            nc.sync.dma_start(out=wq_sb[:, 1], in_=wq.ap()[:, 1])
            nc.sync.dma_start(out=xsl[1][:], in_=xT.ap()[:, :, 512:1024])
            nc.sync.dma_start(out=wq_sb[:, 2], in_=wq.ap()[:, 2])
            nc.sync.dma_start(out=wq_sb[:, 3], in_=wq.ap()[:, 3])
            nc.sync.dma_start(out=xsl[2][:], in_=xT.ap()[:, :, 1024:1536])
            nc.sync.dma_start(out=cos_sb[:], in_=cosT.ap())
            nc.sync.dma_start(out=sin_sb[:], in_=sinTf.ap())
            nc.sync.dma_start(out=xsl[3][:], in_=xT.ap()[:, :, 1536:2048])
            nc.sync.dma_start(out=wo_sb[:], in_=wo.ap())

            for q in range(NSC):
                sl = slice(q * 512, (q + 1) * 512)
                x = xsl[q]
                qraw = rpool.tile([128, 5, 512], BF16, tag="qraw")
                rot = rpool.tile([128, 5, 512], BF16, tag="rot")
                # K projection (transposed layout)
                ps = psA.tile([128, 512], F32, tag="proj", bufs=2)
                for j in range(NE):
                    nc.tensor.matmul(ps[:], wk_sb[:, j, :], x[:, j, :],
                                     start=(j == 0), stop=(j == NE - 1))
                nc.vector.tensor_copy(qraw[:, 0, :], ps[:])
                # V projection directly into natural [s, d] layout
                psv = psA.tile([128, 4, D], F32, tag="vproj", bufs=2)
                for st in range(4):
                    t = q * 4 + st
                    ssl128 = slice(st * 128, (st + 1) * 128)
                    for j in range(NE):
                        nc.tensor.matmul(psv[:, st, :], x[:, j, ssl128],
                                         wv_sb[:, j, :],
                                         start=(j == 0), stop=(j == NE - 1))
                    nc.vector.tensor_copy(vn_sb[:, t, :], psv[:, st, :])
                # Q projections
                for h in range(G):
                    ps = psA.tile([128, 512], F32, tag="proj", bufs=2)
                    for j in range(NE):
                        nc.tensor.matmul(ps[:], wq_sb[:, h, j, :],
                                         x[:, j, :],
                                         start=(j == 0), stop=(j == NE - 1))
                    nc.vector.tensor_copy(qraw[:, 1 + h, :], ps[:])
                # batched rope for all 5 projections of this chunk.
                # rotate_half's partition swap is done by reading the OTHER
                # half's partitions directly (cross-base-partition operands),
                # so no SBUF->SBUF DMA competes with the big input loads.
                cb = _bcast_mid(cos_sb[:, sl], 5)
                nc.vector.tensor_tensor(rot[0:64, :, :], qraw[64:128, :, :],
                                        _bcast_mid(sin_sb[0:64, sl], 5),
                                        OP.mult)
                nc.vector.tensor_tensor(rot[64:128, :, :], qraw[0:64, :, :],
                                        _bcast_mid(sin_sb[64:128, sl], 5),
                                        OP.mult)
                nc.vector.tensor_tensor(qraw[:], qraw[:], cb, OP.mult)
                nc.vector.tensor_tensor(qk_c[q][:], qraw[:], rot[:], OP.add)

        # ================= phase B+C: attention + o_proj interleaved ====
        with (
            tc.tile_pool(name="atp", bufs=3) as atpool,
            tc.tile_pool(name="otp", bufs=2) as otpool,
            tc.tile_pool(name="nrm", bufs=2) as nrmpool,
            tc.tile_pool(name="ost", bufs=2) as opool,
            tc.tile_pool(name="psB", bufs=1, space=bass.MemorySpace.PSUM) as psB,
        ):
            ot_tiles = {}

            def attn_iter(q, h, fast_norm=False, oproj_blocks=()):
                at = atpool.tile([128, NST, 512], BF16, tag="at")
                av = psB.tile([D, 512], F32, tag="av", bufs=2)
                if fast_norm:
                    sm = psB.tile([D, 512], F32, tag="av", bufs=2)
                for tg in range(8):
                    sc2 = psB.tile([128, 2, 512], F32, tag="sc", bufs=2)
                    for tt in range(2):
                        t = 2 * tg + tt
                        nc.tensor.matmul(sc2[:, tt, :],
                                         qk_c[t // 4][:, 0, (t % 4) * 128:
                                                      (t % 4 + 1) * 128],
                                         qk_c[q][:, 1 + h, :],
                                         start=True, stop=True)
                    nc.scalar.activation(at[:, 2 * tg:2 * tg + 2, :], sc2[:],
                                         AF.Exp, scale=SCALE)
                    for tt in range(2):
                        t = 2 * tg + tt
                        nc.tensor.matmul(av[:], vn_sb[:, t, :], at[:, t, :],
                                         start=(t == 0), stop=(t == NST - 1))
                        if fast_norm:
                            nc.tensor.matmul(sm[0:1, :], ones_sb[:, 0:1],
                                             at[:, t, :],
                                             start=(t == 0), stop=(t == NST - 1))
                    # spread o_proj eo-blocks of the previous chunk between
                    # score/av groups so the Act exp pipeline never drains
                    if tg % 2 == 1 and oproj_blocks:
                        oproj_eo(*oproj_blocks[tg // 2])
                ot = otpool.tile([D, 512], BF16, tag=f"ot{h}", name=f"ot{h}_{q}")
                ot_tiles[(q, h)] = ot
                with nc.allow_low_precision(reason="bf16 attention, verified 6e-3 rel err"):
                    nc.vector.tensor_copy(ot[:], av[:])
                    rc = nrmpool.tile([128, 512], BF16, tag="rc")
                    if fast_norm:
                        # last iteration: lowest-latency path so the final
                        # o_proj block is not left waiting on the DVE tree
                        rcr = nrmpool.tile([1, 512], BF16, tag="rcr")
                        nc.vector.reciprocal(rcr[:], sm[0:1, :])
                        bc = psB.tile([D, 512], F32, tag="av", bufs=2)
                        nc.tensor.matmul(bc[:], ones_sb[0:1, :], rcr[:],
                                         start=True, stop=True)
                        nc.vector.tensor_copy(rc[:], bc[:])
                    else:
                        # denominator off PE: wide bf16 tree on DVE
                        # (in-place), cross-partition sum on gpsimd
                        nc.vector.tensor_tensor(at[:, 0:8, :], at[:, 0:8, :],
                                                at[:, 8:16, :], OP.add)
                        nc.vector.tensor_tensor(at[:, 0:4, :], at[:, 0:4, :],
                                                at[:, 4:8, :], OP.add)
                        nc.vector.tensor_tensor(at[:, 0:2, :], at[:, 0:2, :],
                                                at[:, 2:4, :], OP.add)
                        acc = nrmpool.tile([128, 512], BF16, tag="acc")
                        nc.vector.tensor_tensor(acc[:], at[:, 0, :],
                                                at[:, 1, :], OP.add)
                        den = nrmpool.tile([128, 512], F32, tag="den")
                        nc.gpsimd.partition_all_reduce(den[:], acc[:], 128,
                                                       bass_isa.ReduceOp.add)
                        nc.vector.reciprocal(rc[:], den[:])
                    nc.gpsimd.tensor_tensor(ot[:], ot[:], rc[:], OP.mult)

            ostg_tiles = {}

            def oproj_eo(q, st, eo, last=False):
                s0 = q * 512 + st * 128
                if eo == 0:
                    ostg_tiles[st % 2] = opool.tile([128, E], F32, tag="ostg",
                                                    name=f"ostg{q}_{st}")
                ostg = ostg_tiles[st % 2]
                op_ps = psB.tile([128, 512], F32, tag="op", bufs=2)
                for h in range(G):
                    nc.tensor.matmul(
                        op_ps[:],
                        ot_tiles[(q, h)][:, st * 128:(st + 1) * 128],
                        wo_sb[:, h, eo * 512:(eo + 1) * 512],
                        start=(h == 0), stop=(h == G - 1))
                osl = slice(eo * 512, (eo + 1) * 512)
                if last and eo % 2 == 1:
                    nc.scalar.copy(ostg[:, osl], op_ps[:])
                else:
                    nc.vector.tensor_copy(ostg[:, osl], op_ps[:])
                if eo == 1:
                    nc.sync.dma_start(out=out.ap()[s0:s0 + 128, 0:1024],
                                      in_=ostg[:, 0:1024])
                elif eo == 3:
                    nc.sync.dma_start(out=out.ap()[s0:s0 + 128, 1024:2048],
                                      in_=ostg[:, 1024:2048])

            # o_proj eo-blocks of chunk q-1 are interleaved INSIDE chunk q's
            # attention iterations (st index = h) so PE work fills the Act
            # engine's exp latency without ever draining its pipeline.
            for q in range(NSC):
                for h in range(G):
                    blocks = [(q - 1, h, eo) for eo in range(4)] if q >= 1 else ()
                    attn_iter(q, h, fast_norm=(q == NSC - 1 and h == G - 1),
                              oproj_blocks=blocks)
            for st in range(4):
                for eo in range(4):
                    oproj_eo(NSC - 1, st, eo, last=True)


def _build():
    nc = bacc.Bacc("TRN2", target_bir_lowering=False, debug=False,
                   num_devices=NCORES)
    xT = nc.dram_tensor("xT", [128, NE, S], BF16, kind="ExternalInput")
    wq = nc.dram_tensor("wq", [128, G, NE, D], BF16, kind="ExternalInput")
    wk = nc.dram_tensor("wk", [128, NE, D], BF16, kind="ExternalInput")
    wv = nc.dram_tensor("wv", [128, NE, D], BF16, kind="ExternalInput")
    wo = nc.dram_tensor("wo", [128, G, E], BF16, kind="ExternalInput")
    cosT = nc.dram_tensor("cosT", [D, S], BF16, kind="ExternalInput")
    sinTf = nc.dram_tensor("sinTf", [D, S], BF16, kind="ExternalInput")
    out = nc.dram_tensor("out", [S, E], F32, kind="ExternalOutput")
    with tile.TileContext(nc) as tc:
        _emit(nc, tc, xT, wq, wk, wv, wo, cosT, sinTf, out)
    nc.compile()
    return nc


def _rope_tables():
    inv = 1.0 / (ROPE_BASE ** (np.arange(0, D, 2, dtype=np.float64) / D))
    t = np.arange(S, dtype=np.float64)
    freqs = t[:, None] * inv[None, :]                    # [S, D/2]
    emb = np.concatenate([freqs, freqs], axis=-1)        # [S, D]
    cosT = np.cos(emb).T.astype(np.float32)              # [D, S]
    sinT = np.sin(emb).T.astype(np.float32)
    sinTf = sinT.copy()
    sinTf[: D // 2] *= -1.0                              # fold rotate_half sign
    return (np.ascontiguousarray(cosT).astype(ml_dtypes.bfloat16),
            np.ascontiguousarray(sinTf).astype(ml_dtypes.bfloat16))


def _chunked(a, nchunk):
    """[E, F] -> [128, nchunk, F] bf16 with chunk c = rows c*128..(c+1)*128."""
    E_, F_ = a.shape
    return np.ascontiguousarray(
        a.reshape(nchunk, 128, F_).transpose(1, 0, 2)).astype(ml_dtypes.bfloat16)


def _wq_chunked(a):
    """[E, GD] -> [128, G, NE, D] bf16, head-major with 4KB runs."""
    return np.ascontiguousarray(
        a.reshape(NE, 128, G, D).transpose(1, 2, 0, 3)).astype(ml_dtypes.bfloat16)


_NC = None
LAST_RESULTS = None


def kernel(hidden_states, wq, wk, wv, wo):
    global _NC, LAST_RESULTS
    if _NC is None:
        _NC = _build()
    cosT, sinTf = _rope_tables()
    hs = np.asarray(hidden_states, dtype=np.float32)
    wq = np.asarray(wq, dtype=np.float32)
    wk = np.asarray(wk, dtype=np.float32)
    wv = np.asarray(wv, dtype=np.float32)
    wo = np.asarray(wo, dtype=np.float32)

    in_maps = []
    for core in range(NCORES):
        b, g = divmod(core, G)
        in_maps.append({
            "xT": _chunked(np.ascontiguousarray(hs[b].T), NE),
            "wq": _wq_chunked(wq[:, GD * g:GD * (g + 1)]),
            "wk": _chunked(np.ascontiguousarray(wk[:, D * g:D * (g + 1)]), NE),
            "wv": _chunked(np.ascontiguousarray(wv[:, D * g:D * (g + 1)]), NE),
            "wo": _chunked(np.ascontiguousarray(wo[GD * g:GD * (g + 1), :]), G),
            "cosT": cosT,
            "sinTf": sinTf,
        })

    res = run_bass_kernel_spmd(_NC, in_maps, list(range(NCORES)))
    LAST_RESULTS = res
    outs = [np.asarray(res.results[i]["out"], dtype=np.float32)
            for i in range(NCORES)]
    full = np.stack([sum(outs[b * G:(b + 1) * G]) for b in range(B)], axis=0)
    return full.astype(np.float32)
